# revision 8
# baseline (speedup 1.0000x reference)
"""CfC RNN scan kernel for Trainium2 (8 NeuronCores, data-parallel over batch).

Math (per step, from the reference):
    f   = 1.7159 * tanh(0.666 * (concat(x_s, h) @ W0 + b0))     x_s = (x-65)/100
    ff1 = f @ W1 + b1 ;  ff2 = f @ W2 + b2
    ta  = f @ Wa + ba ;  tb  = f @ Wb + bb
    t   = sigmoid(tb - ta * ts)
    h'  = ff1 + t * (ff2 - ff1)

Folding done on the host:
  - input scale/shift folded into W0x, b0:  xterm = x @ (W0x/100) + (b0 - .65*W0x.sum(0))
  - 1.7159 folded into the head weights; heads consume g = tanh(0.666*z) directly
  - d = ff2-ff1 computed via Wd = W2-W1, bd = b2-b1
  - head weights concatenated: Wcat = [W1' | Wd' | Wa' | Wb'] (256 x 512)

End-to-end: the wall-clock here is dominated by the axon tunnel (~60MB/s up,
~35MB/s down), so the runner minimizes wire bytes and overlaps transfers:
  - x ships as fp16 [C+1, S, BL] (half the bytes); the x-term matmuls run in
    fp16 against an fp16 copy of W0aug (error ~1e-5 on z, way under tol).
  - the output ships as fp16 (one ACT downcast per 32-step chunk on-device),
    upcast to f32 on the host. Per-element rel err ~5e-4 vs 2e-2 tolerance.
  - the scan is cut into SEG-step segments chained through a device-resident
    hT state tensor; uploads of segment i+1 and downloads of segment i overlap
    via jax async dispatch + background fetch threads.
  - no 256MB zero-init upload: output placeholder operands are dead (the NEFF
    output binds to the custom-call result buffer), one cached dummy is reused.
  - the jitted executable, device-resident weights, and placeholders are
    cached across calls, so a steady-state call is pure transfer + execute.

On-chip structure (per core, B_local=32):
  - x is fed pre-transposed as xT [C+1, S, BL] fp16 (row C = ones so b0 rides
    the matmul); per 32-step chunk one DMA stages it; per step a matmul pair
    computes the x-dependent backbone term straight into PSUM; the recurrent
    f32 matmul accumulates on top (no eviction/preload).
  - Persistent constants live in a single f32 "blob" (W0h, Wcat, bcat, ones)
    plus a small fp16 W0aug tensor and the f32 h0T state, one DMA each: the
    HW Matmult instruction tolerates a single semaphore wait, so three chained
    1x1 warm-up matmuls absorb the three DMA waits before any real matmul.
  - scan step: hT [128,32] -> MM1 accumulate -> ACT tanh [128,2,32] -> g;
    heads use g as the (P=32) stationary operand: psA=[ta|tb], psB=[ff1|d] in
    separate PSUM banks; per-bank K=1 ones-row matmuls add the biases
    (h-independent, off the critical path).
  - gate: DVE tensor_scalar (ta*-ts, PSUM->SBUF), DVE add (+tb), ACT sigmoid,
    DVE mul (*d), DVE add (+ff1) written into the f32 output staging tile; 4
    DVE 32x32 transposes produce hT for the next step.  At chunk end one ACT
    copy downcasts the staging tile to fp16 for the out DMA.
"""

import sys
import threading
from concurrent.futures import ThreadPoolExecutor

import numpy as np

for _p in ("/opt/trn_rl_repo",):
    if _p not in sys.path:
        sys.path.insert(0, _p)

B, S, C, U, H = 256, 2048, 64, 128, 256
NCORES = 8
BL = B // NCORES  # 32
CHUNK = 32
TS_SUPER = 256  # steps per timespan staging DMA
SEG = 512  # steps per device program (pipeline granularity)

# blob column layout (128 partitions x BLOB_COLS fp32)
_C_W0H = 0            # [128, 256]
_C_WCAT = 256         # [128, 1024] = 2 K-tiles x 512
_C_BC = 1280          # [1, 512] bcat (rows 1..127 stay zero -> zrow)
_C_ONES = 1792        # [1, 32] ones
BLOB_COLS = 1824


def _build_nc(s_total: int):
    import concourse.bass as bass
    import concourse.tile as tile
    from concourse import mybir
    from concourse.tile_rust import add_dep_helper
    import concourse.tile_sem_assignment as _tsa

    # All DMAs go through gpsimd/SWDGE; cap the SWDGE sem count so the
    # kernel-tail Drain's per-queue waits fit its struct's wait slots.
    _tsa.NUM_SWDGE_GLOBAL_SEMS = 2

    f32 = mybir.dt.float32
    f16 = mybir.dt.float16
    AF = mybir.ActivationFunctionType
    nchunk = s_total // CHUNK
    ts_super = min(TS_SUPER, s_total)

    nc = bass.Bass("TRN2")
    xT_d = nc.dram_tensor("xT", [C + 1, s_total, BL], f16, kind="ExternalInput")
    nts_d = nc.dram_tensor("nts", [BL, s_total], f32, kind="ExternalInput")
    blob_d = nc.dram_tensor("blob", [128, BLOB_COLS], f32, kind="ExternalInput")
    w0a_d = nc.dram_tensor("w0a", [C + 1, H], f16, kind="ExternalInput")
    h0T_d = nc.dram_tensor("h0T", [U, BL], f32, kind="ExternalInput")
    out_d = nc.dram_tensor("out", [BL, s_total, U], f16, kind="ExternalOutput")
    hTo_d = nc.dram_tensor("hTout", [U, BL], f32, kind="ExternalOutput")

    with tile.TileContext(nc) as tc:
        with (
            tc.tile_pool(name="singles", bufs=1) as singles,
            tc.tile_pool(name="xstage", bufs=2) as xstage,
            tc.tile_pool(name="tsstage", bufs=2) as tsstage,
            tc.tile_pool(name="outstage16", bufs=2) as outstage16,
            tc.tile_pool(name="ft", bufs=6) as ftp,
            tc.tile_pool(name="fb", bufs=6) as fbp,
            tc.tile_pool(name="gate", bufs=6) as gatep,
            tc.tile_pool(name="nh", bufs=3) as nhp,
            tc.tile_pool(name="ht", bufs=2) as htp,
            tc.tile_pool(name="psf", bufs=3, space="PSUM") as psfp,
            tc.tile_pool(name="psbnd", bufs=1, space="PSUM") as psbndp,
            tc.tile_pool(name="psa", bufs=2, space="PSUM") as psap,
            tc.tile_pool(name="psb", bufs=2, space="PSUM") as psbp,
        ):
            sb_blob = singles.tile([128, BLOB_COLS], f32, tag="blob")
            nc.gpsimd.dma_start(out=sb_blob, in_=blob_d[:, :])
            sb_w0a = singles.tile([C + 1, H], f16, tag="w0a")
            nc.gpsimd.dma_start(out=sb_w0a, in_=w0a_d[:, :])
            sb_h0T = singles.tile([U, BL], f32, tag="h0T")
            nc.gpsimd.dma_start(out=sb_h0T, in_=h0T_d[:, :])

            sb_W0h = sb_blob[:, _C_W0H : _C_W0H + H]
            sb_scr = singles.tile([1, 16], f32, tag="scratch")
            # a zero row of the blob: row 64 of the bcat column range (only
            # row 0 holds data there); base partition must be 0/32/64
            sb_zrow = sb_blob[64:65, _C_BC : _C_BC + 256]
            sb_bcat = sb_blob[0:1, _C_BC : _C_BC + 4 * U]
            sb_ones = sb_blob[0:1, _C_ONES : _C_ONES + BL]

            def wcat(k2, lo, hi):
                base = _C_WCAT + k2 * 4 * U
                return sb_blob[:, base + lo : base + hi]

            # warm-up: three 1x1 matmuls so PE observes each input DMA's
            # semaphore before any real matmul (Matmult carries at most one
            # sync wait); PE is in-order so they need no inter-deps.
            ps_w = psap.tile([BL, 2 * U], f32, tag="psa")
            nc.tensor.matmul(
                ps_w[0:1, 0:1], sb_blob[0:1, 0:1], sb_blob[0:1, 0:1],
                start=True, stop=True,
            )
            nc.tensor.matmul(
                ps_w[0:1, 0:1], sb_w0a[0:1, 0:1], sb_w0a[0:1, 0:1],
                start=True, stop=True,
            )
            nc.tensor.matmul(
                ps_w[0:1, 0:1], sb_h0T[0:1, 0:1], sb_h0T[0:1, 0:1],
                start=True, stop=True,
            )

            cur_hT = sb_h0T
            prev_pe = None  # last PE instruction of the previous step
            prev_act = None  # nosync chain pinning the ACT instruction order

            for ci in range(nchunk):
                s0 = ci * CHUNK
                xTa = xstage.tile([C + 1, CHUNK * BL], f16, tag="xta")
                nc.gpsimd.dma_start(out=xTa, in_=xT_d[:, s0 : s0 + CHUNK, :])
                if s0 % ts_super == 0:
                    ntss = tsstage.tile([BL, ts_super], f32, tag="ntss")
                    nc.gpsimd.dma_start(out=ntss, in_=nts_d[:, s0 : s0 + ts_super])
                    # DVE toucher: absorb the ntss DMA wait on DVE once, so
                    # per-step tensor_scalar ops don't carry a second wait
                    sci = s0 // ts_super
                    nc.vector.tensor_copy(
                        sb_scr[0:1, sci : sci + 1], ntss[0:1, 0:1]
                    )

                ostage16 = outstage16.tile([BL, CHUNK * U], f16, tag="ostage16")
                # DVE toucher: absorb the WAR on the previous out-DMA of this
                # staging buffer so the per-step fp16 copies have only one wait
                nc.vector.memset(ostage16[0:1, 0:1], 0.0)

                for s in range(CHUNK):
                    st = (s0 + s) % ts_super  # index into ntss
                    # backbone: z = x-term + W0h.T @ hT, one accumulation group
                    # per m-tile (the x-term matmul is h-independent and runs
                    # ahead; same-group accumulation avoids extra PE waits)
                    # chunk-boundary step uses a dedicated psum tile: its
                    # slot-reuse WAW wait is then chunk-distant (dominated),
                    # leaving room for the xTa DMA wait (1-wait limit)
                    if s == 0:
                        ps_f = psbndp.tile([128, 2, BL], f32, tag="psbnd")
                    else:
                        ps_f = psfp.tile([128, 2, BL], f32, tag="psf")
                    # start=True clears the ENTIRE psum bank, so the two
                    # m-tiles (sharing one bank) must not each lead their own
                    # group: one K=1 zero-matmul clears/claims the whole
                    # region, everything else accumulates.
                    clr = nc.tensor.matmul(
                        ps_f,
                        sb_zrow[:, 0:128],
                        sb_zrow[:, 0 : 2 * BL],
                        start=True,
                        stop=False,
                        skip_group_check=True,
                    )
                    if prev_pe is not None:
                        add_dep_helper(clr.ins, prev_pe.ins, False, "clr after heads")
                    for m in range(2):
                        nc.tensor.matmul(
                            ps_f[:, m, :],
                            sb_w0a[:, m * 128 : (m + 1) * 128],
                            xTa[:, s * BL : (s + 1) * BL],
                            start=False,
                            stop=False,
                            skip_group_check=True,
                        )
                    mm1_last = None
                    for m in range(2):
                        mm1_last = nc.tensor.matmul(
                            ps_f[:, m, :],
                            sb_W0h[:, m * 128 : (m + 1) * 128],
                            cur_hT,
                            start=False,
                            stop=True,
                            skip_group_check=True,
                        )
                    # g = tanh(0.666 * z), both H-tiles in one ACT op
                    fT = ftp.tile([128, 2, BL], f32, tag="ft")
                    th = nc.scalar.activation(fT, ps_f, AF.Tanh, scale=0.666)
                    if prev_act is not None:
                        # nosync chain: fixes the ACT stream order so slot
                        # reuse stays outside the queue window and no ACT
                        # self-waits are emitted (Activation has 1 wait slot)
                        add_dep_helper(th.ins, prev_act.ins, False, "act chain")
                    prev_act = th

                    # heads: psA = [ta | tb], psB = [ff1 | d] (separate banks)
                    psA = psap.tile([BL, 2 * U], f32, tag="psa")
                    psB = psbp.tile([BL, 2 * U], f32, tag="psb")
                    # order-only dep: keep the bias matmuls behind this
                    # step's MM1 so their psum-WAR wait is dominated by MM1's
                    # DVE wait (Matmult tolerates only one sync wait)
                    bmA = nc.tensor.matmul(
                        psA, sb_ones, sb_bcat[:, 2 * U : 4 * U], start=True, stop=False
                    )
                    bmB = nc.tensor.matmul(
                        psB, sb_ones, sb_bcat[:, 0 : 2 * U], start=True, stop=False
                    )
                    add_dep_helper(bmA.ins, mm1_last.ins, False, "bias after MM1")
                    add_dep_helper(bmB.ins, mm1_last.ins, False, "bias after MM1")
                    for k2 in range(2):
                        nc.tensor.matmul(
                            psA,
                            fT[:, k2, :],
                            wcat(k2, 2 * U, 4 * U),
                            start=False,
                            stop=(k2 == 1),
                        )
                    for k2 in range(2):
                        prev_pe = nc.tensor.matmul(
                            psB,
                            fT[:, k2, :],
                            wcat(k2, 0, 2 * U),
                            start=False,
                            stop=(k2 == 1),
                        )

                    # gate: v = tb - ta*ts ; t = sigmoid(v) ; nh = ff1 + t*d
                    # (only one PSUM input allowed per DVE op). psB is evicted
                    # to SBUF on ACT (hidden behind t1/v) so t3's single ACT
                    # wait covers both the sigmoid and [ff1|d].
                    t1 = gatep.tile([BL, U], f32, tag="t1")
                    nc.vector.tensor_scalar_mul(t1, psA[:, 0:U], ntss[:, st : st + 1])
                    v = gatep.tile([BL, U], f32, tag="v")
                    nc.vector.tensor_add(v, t1, psA[:, U : 2 * U])
                    fB = fbp.tile([BL, 2 * U], f32, tag="fb")
                    cb = nc.scalar.copy(fB, psB)
                    add_dep_helper(cb.ins, prev_act.ins, False, "act chain")
                    prev_act = cb
                    sg = gatep.tile([BL, U], f32, tag="sg")
                    sgi = nc.scalar.activation(sg, v, AF.Sigmoid)
                    add_dep_helper(sgi.ins, prev_act.ins, False, "act chain")
                    prev_act = sgi
                    t3 = gatep.tile([BL, U], f32, tag="t3")
                    nc.vector.tensor_mul(t3, sg, fB[:, U : 2 * U])
                    nh = nhp.tile([BL, U], f32, tag="nh")
                    nc.vector.tensor_add(nh, t3, fB[:, 0:U])
                    # fp16 downcast into the output staging tile (DVE copy)
                    nc.vector.tensor_copy(
                        ostage16[:, s * U : (s + 1) * U], nh
                    )

                    # hT for the next step: 4x 32x32 DVE transposes
                    hT = htp.tile([U, BL], f32, tag="ht")
                    for j in range(4):
                        nc.vector.transpose(
                            hT[32 * j : 32 * (j + 1), :],
                            nh[:, 32 * j : 32 * (j + 1)],
                        )
                    cur_hT = hT

                nc.gpsimd.dma_start(out=out_d[:, s0 : s0 + CHUNK, :], in_=ostage16)

            # final hidden state (transposed) for segment chaining
            nc.gpsimd.dma_start(out=hTo_d[:, :], in_=cur_hT)

    _drop_stale_self_waits(nc, mybir)
    return nc


def _drop_stale_self_waits(nc, mybir, margin=8):
    """Compute instructions have a single usable wait slot (the engine-sem
    update takes the other).  Tile emits same-engine/same-lane waits for
    slot reuse even when the producer is far back; on an in-order engine or
    FIFO DMA lane those are redundant.  Drop self waits on instructions
    carrying >1 wait: engine-sem waits when >= `margin` instructions stale,
    own-DMA-lane waits always (the lane is FIFO)."""
    eng_prefix = {
        mybir.EngineType.PE: "PE",
        mybir.EngineType.DVE: "DVE",
        mybir.EngineType.Activation: "Activation",
        mybir.EngineType.Pool: "Pool",
        mybir.EngineType.SP: "SP",
    }
    tick = {}
    eng_ic = {}  # engine -> instruction count so far
    reach = {}  # sem name -> list of (value, engine_instr_idx) in order
    for fn in nc.m.functions:
        for blk in fn.blocks:
            for i in blk.instructions:
                si = i.sync_info
                if si is None:
                    continue
                eng = getattr(i, "engine", None)
                pfx = eng_prefix.get(eng)
                my_ic = eng_ic.get(eng, 0)
                upd_sems = {u.ant_name for u in si.on_update}
                if len(si.on_wait) > 1:
                    is_dma = type(i).__name__ == "InstDMACopy"
                    kept = []
                    for w in si.on_wait:
                        n = w.ant_name
                        if pfx and n.startswith(pfx + "_"):
                            # same-engine self-wait: staleness measured in
                            # engine-instruction distance (sem values are
                            # assigned round-robin, so value distance lies)
                            hist = reach.get(n, [])
                            prod_ic = None
                            for v, ic in reversed(hist):
                                if v >= w.wait_value:
                                    prod_ic = ic
                                else:
                                    break
                            if prod_ic is not None and my_ic - prod_ic >= margin:
                                continue  # stale engine self-wait
                        if (
                            is_dma
                            and n in upd_sems
                            and ("DMASW" in n or "DMAHW" in n)
                            and tick.get(n, 0) >= w.wait_value
                        ):
                            continue  # own-lane FIFO wait
                        kept.append(w)
                    if len(kept) != len(si.on_wait):
                        si.on_wait = kept
                for u in si.on_update:
                    v = tick.get(u.ant_name, 0) + u.update_value
                    tick[u.ant_name] = v
                    reach.setdefault(u.ant_name, []).append((v, my_ic))
                eng_ic[eng] = my_ic + 1
    _split_multiwait_drains(nc, mybir)


def _split_multiwait_drains(nc, mybir):
    """The kernel-tail Drain waits on every engine/DMA-lane sem, but its
    struct has a single wait slot.  Split: inject one single-wait Drain per
    extra wait immediately before it on the same engine."""
    for fn in nc.m.functions:
        for blk in fn.blocks:
            insts = blk.instructions
            out = []
            changed = False
            for i in insts:
                si = i.sync_info
                if type(i).__name__ == "InstDrain" and si and len(si.on_wait) > 1:
                    waits = list(si.on_wait)
                    for k, w in enumerate(waits[:-1]):
                        d = mybir.InstDrain(name=f"{i.name}-w{k}", ins=[], outs=[])
                        d.engine = i.engine
                        d.sync_info = mybir.SyncInfo(on_wait=[w], on_update=[])
                        out.append(d)
                    si.on_wait = [waits[-1]]
                    changed = True
                out.append(i)
            if changed:
                blk.instructions = out


def _prep_weights(W0, b0, W1, b1, W2, b2, Wa, ba, Wb, bb):
    W0 = np.asarray(W0, np.float32)
    W0x = W0[:C] / 100.0
    W0h = np.ascontiguousarray(W0[C:])  # [U, H]
    b0p = np.asarray(b0, np.float32) - 0.65 * W0[:C].sum(axis=0)
    W0aug = np.concatenate([W0x, b0p[None, :]], axis=0)  # [C+1, H]
    a = np.float32(1.7159)
    Wcat = np.concatenate([a * W1, a * (W2 - W1), a * Wa, a * Wb], axis=1)  # [H, 4U]
    bcat = np.concatenate([b1, b2 - b1, ba, bb]).astype(np.float32)  # [4U]
    return (
        W0aug.astype(np.float32),
        W0h.astype(np.float32),
        Wcat.astype(np.float32),
        bcat,
    )


def _make_blob(weights):
    W0aug, W0h, Wcat, bcat = weights
    blob = np.zeros((128, BLOB_COLS), np.float32)
    blob[:, _C_W0H : _C_W0H + H] = W0h
    for k2 in range(2):
        blob[:, _C_WCAT + k2 * 4 * U : _C_WCAT + (k2 + 1) * 4 * U] = Wcat[
            k2 * 128 : (k2 + 1) * 128, :
        ]
    blob[0, _C_BC : _C_BC + 4 * U] = bcat
    blob[0, _C_ONES : _C_ONES + BL] = 1.0
    return blob


class _Res:
    exec_time_ns = None
    mean_exec_time_ns = None
    instructions_and_trace = None
    profile_json = None


_CACHE = {}
_CACHE_LOCK = threading.Lock()


def _get_rt(s_seg):
    """Build (once) the Bass program + jitted sharded callable for a segment
    length, plus cached device-resident output placeholders."""
    key = ("rt", s_seg)
    with _CACHE_LOCK:
        if key in _CACHE:
            return _CACHE[key]
    import jax
    from jax.sharding import Mesh, PartitionSpec, NamedSharding
    from jax.experimental.shard_map import shard_map
    from concourse import mybir
    from concourse.bass2jax import (
        _bass_exec_p,
        install_neuronx_cc_hook,
        partition_id_tensor,
    )

    install_neuronx_cc_hook()
    nc = _build_nc(s_seg)

    in_names, out_names, out_avals = [], [], []
    for alloc in nc.m.functions[0].allocations:
        if not isinstance(alloc, mybir.MemoryLocationSet):
            continue
        name = alloc.memorylocations[0].name
        if alloc.kind == "ExternalInput":
            in_names.append(name)
        elif alloc.kind == "ExternalOutput":
            out_names.append(name)
            out_avals.append(
                jax.core.ShapedArray(
                    tuple(alloc.tensor_shape), mybir.dt.np(alloc.dtype)
                )
            )
    partition_name = nc.partition_id_tensor.name if nc.partition_id_tensor else None
    if partition_name is not None:
        in_names.remove(partition_name)
    all_in = in_names + out_names

    def _body(*args):
        operands = list(args)
        if partition_name is not None:
            operands.append(partition_id_tensor())
        outs = _bass_exec_p.bind(
            *operands,
            out_avals=tuple(out_avals),
            in_names=tuple(all_in + ([partition_name] if partition_name else [])),
            out_names=tuple(out_names),
            lowering_input_output_aliases=(),
            sim_require_finite=True,
            sim_require_nnan=True,
            nc=nc,
        )
        return tuple(outs)

    devices = jax.devices()[:NCORES]
    mesh = Mesh(np.asarray(devices), ("core",))
    P = PartitionSpec
    jitted = jax.jit(
        shard_map(
            _body,
            mesh=mesh,
            in_specs=(P("core"),) * len(all_in),
            out_specs=(P("core"),) * len(out_names),
            check_rep=False,
        ),
        keep_unused=True,
    )
    sh = NamedSharding(mesh, P("core"))
    ph_out = jax.device_put(np.zeros((NCORES * BL, s_seg, U), np.float16), sh)
    ph_hT = jax.device_put(np.zeros((NCORES * U, BL), np.float32), sh)
    rt = dict(
        nc=nc, jitted=jitted, sh=sh, in_names=in_names, out_names=out_names,
        ph_out=ph_out, ph_hT=ph_hT,
    )
    with _CACHE_LOCK:
        _CACHE[key] = rt
    return rt


def _weights_dev(weights, sh):
    """Device-resident replicated weight tensors, cached per weights object."""
    import jax

    key = ("wdev", id(weights))
    with _CACHE_LOCK:
        hit = _CACHE.get(key)
    if hit is not None:
        return hit
    blob = _make_blob(weights)  # [128, BLOB_COLS] f32
    w0a16 = weights[0].astype(np.float16)  # [C+1, H]
    blob_g = np.broadcast_to(blob, (NCORES, *blob.shape)).reshape(
        NCORES * 128, BLOB_COLS
    )
    w0a_g = np.broadcast_to(w0a16, (NCORES, *w0a16.shape)).reshape(
        NCORES * (C + 1), H
    )
    blob_dev = jax.device_put(np.ascontiguousarray(blob_g), sh)
    w0a_dev = jax.device_put(np.ascontiguousarray(w0a_g), sh)
    val = (blob_dev, w0a_dev)
    with _CACHE_LOCK:
        _CACHE[key] = val
    return val


def _prep_x_seg(x16, s0, s_seg):
    """[B, S, C] f16 -> per-core-concat [8*(C+1), s_seg, BL] f16."""
    xg = np.empty((NCORES * (C + 1), s_seg, BL), np.float16)
    for c in range(NCORES):
        blk = xg[c * (C + 1) : (c + 1) * (C + 1)]
        blk[:C] = x16[c * BL : (c + 1) * BL, s0 : s0 + s_seg, :].transpose(2, 1, 0)
        blk[C] = np.float16(1.0)
    return xg


def _fetch_shard(full, s0, s_seg, shard):
    c = shard.index[0].start // BL
    arr = np.asarray(shard.data)  # [BL, s_seg, U] f16 (blocks until ready)
    full[c * BL : (c + 1) * BL, s0 : s0 + s_seg] = arr  # f16 -> f32 cast


def run(x_codes, h0, timespans, weights, s_total=S, trace=False):
    import jax

    s_seg = min(SEG, s_total)
    assert s_total % s_seg == 0 and s_seg % CHUNK == 0
    nseg = s_total // s_seg
    rt = _get_rt(s_seg)
    blob_dev, w0a_dev = _weights_dev(weights, rt["sh"])

    x16 = np.asarray(x_codes, np.float32)[:, :s_total].astype(np.float16)
    nts = -np.asarray(timespans, np.float32)[:, :s_total]
    h0 = np.asarray(h0, np.float32)
    h0T_g = np.ascontiguousarray(
        h0.reshape(NCORES, BL, U).transpose(0, 2, 1).reshape(NCORES * U, BL)
    )
    h_cur = jax.device_put(h0T_g, rt["sh"])

    by_name_static = {"blob": blob_dev, "w0a": w0a_dev}
    full = np.empty((B, s_total, U), np.float32)
    futs = []
    with ThreadPoolExecutor(max_workers=8) as ex:
        for si in range(nseg):
            s0 = si * s_seg
            xg = _prep_x_seg(x16, s0, s_seg)
            ng = np.ascontiguousarray(nts[:, s0 : s0 + s_seg])
            x_dev = jax.device_put(xg, rt["sh"])
            n_dev = jax.device_put(ng, rt["sh"])
            by_name = {
                **by_name_static,
                "xT": x_dev, "nts": n_dev, "h0T": h_cur,
                "out": rt["ph_out"], "hTout": rt["ph_hT"],
            }
            args = [by_name[n] for n in rt["in_names"] + rt["out_names"]]
            res = rt["jitted"](*args)
            outs = dict(zip(rt["out_names"], res))
            h_cur = outs["hTout"]
            for shard in outs["out"].addressable_shards:
                futs.append(ex.submit(_fetch_shard, full, s0, s_seg, shard))
        for f in futs:
            f.result()
    return full, _Res()


def kernel(x_codes, h0, timespans, W0, b0, W1, b1, W2, b2, Wa, ba, Wb, bb):
    weights = _prep_weights(W0, b0, W1, b1, W2, b2, Wa, ba, Wb, bb)
    full, _ = run(
        np.asarray(x_codes, np.float32),
        np.asarray(h0, np.float32),
        np.asarray(timespans, np.float32),
        weights,
        S,
    )
    return full.astype(np.float32)


# revision 10
# speedup vs baseline: 1.1247x; 1.1247x over previous
"""CfC RNN scan kernel for Trainium2 (8 NeuronCores, data-parallel over batch).

Math (per step, from the reference):
    f   = 1.7159 * tanh(0.666 * (concat(x_s, h) @ W0 + b0))     x_s = (x-65)/100
    ff1 = f @ W1 + b1 ;  ff2 = f @ W2 + b2
    ta  = f @ Wa + ba ;  tb  = f @ Wb + bb
    t   = sigmoid(tb - ta * ts)
    h'  = ff1 + t * (ff2 - ff1)

Folding done on the host:
  - input scale/shift folded into W0x, b0:  xterm = x @ (W0x/100) + (b0 - .65*W0x.sum(0))
  - 1.7159 folded into the head weights; heads consume g = tanh(0.666*z) directly
  - d = ff2-ff1 computed via Wd = W2-W1, bd = b2-b1
  - head weights concatenated: Wcat = [W1' | Wd' | Wa' | Wb'] (256 x 512)

End-to-end: the wall-clock here is dominated by the axon tunnel (~60MB/s up,
~35MB/s down), so the runner minimizes wire bytes and overlaps transfers:
  - x ships as fp16 [C+1, S, BL] (half the bytes); the x-term matmuls run in
    fp16 against an fp16 copy of W0aug (error ~1e-5 on z, way under tol).
  - the output ships as fp16 (one ACT downcast per 32-step chunk on-device),
    upcast to f32 on the host. Per-element rel err ~5e-4 vs 2e-2 tolerance.
  - the scan is cut into SEG-step segments chained through a device-resident
    hT state tensor; uploads of segment i+1 and downloads of segment i overlap
    via jax async dispatch + background fetch threads.
  - no 256MB zero-init upload: output placeholder operands are dead (the NEFF
    output binds to the custom-call result buffer), one cached dummy is reused.
  - the jitted executable, device-resident weights, and placeholders are
    cached across calls, so a steady-state call is pure transfer + execute.

On-chip structure (per core, B_local=32):
  - x is fed pre-transposed as xT [C+1, S, BL] fp16 (row C = ones so b0 rides
    the matmul); per 32-step chunk one DMA stages it; per step a matmul pair
    computes the x-dependent backbone term straight into PSUM; the recurrent
    f32 matmul accumulates on top (no eviction/preload).
  - Persistent constants live in a single f32 "blob" (W0h, Wcat, bcat, ones)
    plus a small fp16 W0aug tensor and the f32 h0T state, one DMA each: the
    HW Matmult instruction tolerates a single semaphore wait, so three chained
    1x1 warm-up matmuls absorb the three DMA waits before any real matmul.
  - scan step: hT [128,32] -> MM1 accumulate -> ACT tanh [128,2,32] -> g;
    heads use g as the (P=32) stationary operand: psA=[ta|tb], psB=[ff1|d] in
    separate PSUM banks; per-bank K=1 ones-row matmuls add the biases
    (h-independent, off the critical path).
  - gate: DVE tensor_scalar (ta*-ts, PSUM->SBUF), DVE add (+tb), ACT sigmoid,
    DVE mul (*d), DVE add (+ff1) written into the f32 output staging tile; 4
    DVE 32x32 transposes produce hT for the next step.  At chunk end one ACT
    copy downcasts the staging tile to fp16 for the out DMA.
"""

import sys
import threading
from concurrent.futures import ThreadPoolExecutor

import numpy as np

for _p in ("/opt/trn_rl_repo",):
    if _p not in sys.path:
        sys.path.insert(0, _p)

B, S, C, U, H = 256, 2048, 64, 128, 256
NCORES = 8
BL = B // NCORES  # 32
CHUNK = 32
TS_SUPER = 256  # steps per timespan staging DMA
SEG = 512  # steps per device program (pipeline granularity)

# blob column layout (128 partitions x BLOB_COLS fp32)
_C_W0H = 0            # [128, 256]
_C_WCAT = 256         # [128, 1024] = 2 K-tiles x 512
_C_BC = 1280          # [1, 512] bcat (rows 1..127 stay zero -> zrow)
_C_ONES = 1792        # [1, 32] ones
BLOB_COLS = 1824


def _build_nc(s_total: int):
    import concourse.bass as bass
    import concourse.tile as tile
    from concourse import mybir
    from concourse.tile_rust import add_dep_helper
    import concourse.tile_sem_assignment as _tsa

    # All DMAs go through gpsimd/SWDGE; cap the SWDGE sem count so the
    # kernel-tail Drain's per-queue waits fit its struct's wait slots.
    _tsa.NUM_SWDGE_GLOBAL_SEMS = 2

    f32 = mybir.dt.float32
    f16 = mybir.dt.float16
    AF = mybir.ActivationFunctionType
    nchunk = s_total // CHUNK
    ts_super = min(TS_SUPER, s_total)

    nc = bass.Bass("TRN2")
    i8 = mybir.dt.int8
    xT_d = nc.dram_tensor("xT", [C + 1, s_total, BL], i8, kind="ExternalInput")
    nts_d = nc.dram_tensor("nts", [BL, s_total], f32, kind="ExternalInput")
    blob_d = nc.dram_tensor("blob", [128, BLOB_COLS], f32, kind="ExternalInput")
    w0a_d = nc.dram_tensor("w0a", [C + 1, H], f16, kind="ExternalInput")
    h0T_d = nc.dram_tensor("h0T", [U, BL], f32, kind="ExternalInput")
    out_d = nc.dram_tensor("out", [BL, s_total, U], f16, kind="ExternalOutput")
    hTo_d = nc.dram_tensor("hTout", [U, BL], f32, kind="ExternalOutput")

    with tile.TileContext(nc) as tc:
        with (
            tc.tile_pool(name="singles", bufs=1) as singles,
            tc.tile_pool(name="xstage", bufs=2) as xstage,
            tc.tile_pool(name="xfst", bufs=2) as xfstage,
            tc.tile_pool(name="tsstage", bufs=2) as tsstage,
            tc.tile_pool(name="outstage16", bufs=2) as outstage16,
            tc.tile_pool(name="ft", bufs=6) as ftp,
            tc.tile_pool(name="fb", bufs=6) as fbp,
            tc.tile_pool(name="gate", bufs=6) as gatep,
            tc.tile_pool(name="nh", bufs=3) as nhp,
            tc.tile_pool(name="ht", bufs=2) as htp,
            tc.tile_pool(name="psf", bufs=3, space="PSUM") as psfp,
            tc.tile_pool(name="psbnd", bufs=1, space="PSUM") as psbndp,
            tc.tile_pool(name="psa", bufs=2, space="PSUM") as psap,
            tc.tile_pool(name="psb", bufs=2, space="PSUM") as psbp,
        ):
            sb_blob = singles.tile([128, BLOB_COLS], f32, tag="blob")
            nc.gpsimd.dma_start(out=sb_blob, in_=blob_d[:, :])
            sb_w0a = singles.tile([C + 1, H], f16, tag="w0a")
            nc.gpsimd.dma_start(out=sb_w0a, in_=w0a_d[:, :])
            sb_h0T = singles.tile([U, BL], f32, tag="h0T")
            nc.gpsimd.dma_start(out=sb_h0T, in_=h0T_d[:, :])

            sb_W0h = sb_blob[:, _C_W0H : _C_W0H + H]
            sb_scr = singles.tile([1, 16], f32, tag="scratch")
            # a zero row of the blob: row 64 of the bcat column range (only
            # row 0 holds data there); base partition must be 0/32/64
            sb_zrow = sb_blob[64:65, _C_BC : _C_BC + 256]
            sb_bcat = sb_blob[0:1, _C_BC : _C_BC + 4 * U]
            sb_ones = sb_blob[0:1, _C_ONES : _C_ONES + BL]

            def wcat(k2, lo, hi):
                base = _C_WCAT + k2 * 4 * U
                return sb_blob[:, base + lo : base + hi]

            # warm-up: three 1x1 matmuls so PE observes each input DMA's
            # semaphore before any real matmul (Matmult carries at most one
            # sync wait); PE is in-order so they need no inter-deps.
            ps_w = psap.tile([BL, 2 * U], f32, tag="psa")
            nc.tensor.matmul(
                ps_w[0:1, 0:1], sb_blob[0:1, 0:1], sb_blob[0:1, 0:1],
                start=True, stop=True,
            )
            nc.tensor.matmul(
                ps_w[0:1, 0:1], sb_w0a[0:1, 0:1], sb_w0a[0:1, 0:1],
                start=True, stop=True,
            )
            nc.tensor.matmul(
                ps_w[0:1, 0:1], sb_h0T[0:1, 0:1], sb_h0T[0:1, 0:1],
                start=True, stop=True,
            )

            cur_hT = sb_h0T
            prev_pe = None  # last PE instruction of the previous step
            prev_act = None  # nosync chain pinning the ACT instruction order

            for ci in range(nchunk):
                s0 = ci * CHUNK
                xTa8 = xstage.tile([C + 1, CHUNK * BL], i8, tag="xta8")
                nc.gpsimd.dma_start(out=xTa8, in_=xT_d[:, s0 : s0 + CHUNK, :])
                xTa = xfstage.tile([C + 1, CHUNK * BL], f16, tag="xta")
                # ACT toucher: absorb the WAR vs this buffer's PE readers two
                # chunks back, so the convert below carries only the DMA wait
                xt_t = nc.scalar.copy(xTa[0:1, 0:1], sb_blob[0:1, 0:1])
                if prev_act is not None:
                    add_dep_helper(xt_t.ins, prev_act.ins, False, "act chain")
                prev_act = xt_t
                # dequantize int8 -> fp16 (scale is folded into w0a host-side)
                xt_c = nc.scalar.copy(xTa, xTa8)
                add_dep_helper(xt_c.ins, prev_act.ins, False, "act chain")
                prev_act = xt_c
                if s0 % ts_super == 0:
                    ntss = tsstage.tile([BL, ts_super], f32, tag="ntss")
                    nc.gpsimd.dma_start(out=ntss, in_=nts_d[:, s0 : s0 + ts_super])
                    # DVE toucher: absorb the ntss DMA wait on DVE once, so
                    # per-step tensor_scalar ops don't carry a second wait
                    sci = s0 // ts_super
                    nc.vector.tensor_copy(
                        sb_scr[0:1, sci : sci + 1], ntss[0:1, 0:1]
                    )

                ostage16 = outstage16.tile([BL, CHUNK * U], f16, tag="ostage16")
                # DVE toucher: absorb the WAR on the previous out-DMA of this
                # staging buffer so the per-step fp16 copies have only one wait
                nc.vector.memset(ostage16[0:1, 0:1], 0.0)

                for s in range(CHUNK):
                    st = (s0 + s) % ts_super  # index into ntss
                    # backbone: z = x-term + W0h.T @ hT, one accumulation group
                    # per m-tile (the x-term matmul is h-independent and runs
                    # ahead; same-group accumulation avoids extra PE waits)
                    # chunk-boundary step uses a dedicated psum tile: its
                    # slot-reuse WAW wait is then chunk-distant (dominated),
                    # leaving room for the xTa DMA wait (1-wait limit)
                    if s == 0:
                        ps_f = psbndp.tile([128, 2, BL], f32, tag="psbnd")
                    else:
                        ps_f = psfp.tile([128, 2, BL], f32, tag="psf")
                    # start=True clears the ENTIRE psum bank, so the two
                    # m-tiles (sharing one bank) must not each lead their own
                    # group: one K=1 zero-matmul clears/claims the whole
                    # region, everything else accumulates.
                    clr = nc.tensor.matmul(
                        ps_f,
                        sb_zrow[:, 0:128],
                        sb_zrow[:, 0 : 2 * BL],
                        start=True,
                        stop=False,
                        skip_group_check=True,
                    )
                    if prev_pe is not None:
                        add_dep_helper(clr.ins, prev_pe.ins, False, "clr after heads")
                    for m in range(2):
                        nc.tensor.matmul(
                            ps_f[:, m, :],
                            sb_w0a[:, m * 128 : (m + 1) * 128],
                            xTa[:, s * BL : (s + 1) * BL],
                            start=False,
                            stop=False,
                            skip_group_check=True,
                        )
                    mm1_last = None
                    for m in range(2):
                        mm1_last = nc.tensor.matmul(
                            ps_f[:, m, :],
                            sb_W0h[:, m * 128 : (m + 1) * 128],
                            cur_hT,
                            start=False,
                            stop=True,
                            skip_group_check=True,
                        )
                    # g = tanh(0.666 * z), both H-tiles in one ACT op
                    fT = ftp.tile([128, 2, BL], f32, tag="ft")
                    th = nc.scalar.activation(fT, ps_f, AF.Tanh, scale=0.666)
                    if prev_act is not None:
                        # nosync chain: fixes the ACT stream order so slot
                        # reuse stays outside the queue window and no ACT
                        # self-waits are emitted (Activation has 1 wait slot)
                        add_dep_helper(th.ins, prev_act.ins, False, "act chain")
                    prev_act = th

                    # heads: psA = [ta | tb], psB = [ff1 | d] (separate banks)
                    psA = psap.tile([BL, 2 * U], f32, tag="psa")
                    psB = psbp.tile([BL, 2 * U], f32, tag="psb")
                    # order-only dep: keep the bias matmuls behind this
                    # step's MM1 so their psum-WAR wait is dominated by MM1's
                    # DVE wait (Matmult tolerates only one sync wait)
                    bmA = nc.tensor.matmul(
                        psA, sb_ones, sb_bcat[:, 2 * U : 4 * U], start=True, stop=False
                    )
                    bmB = nc.tensor.matmul(
                        psB, sb_ones, sb_bcat[:, 0 : 2 * U], start=True, stop=False
                    )
                    add_dep_helper(bmA.ins, mm1_last.ins, False, "bias after MM1")
                    add_dep_helper(bmB.ins, mm1_last.ins, False, "bias after MM1")
                    for k2 in range(2):
                        nc.tensor.matmul(
                            psA,
                            fT[:, k2, :],
                            wcat(k2, 2 * U, 4 * U),
                            start=False,
                            stop=(k2 == 1),
                        )
                    for k2 in range(2):
                        prev_pe = nc.tensor.matmul(
                            psB,
                            fT[:, k2, :],
                            wcat(k2, 0, 2 * U),
                            start=False,
                            stop=(k2 == 1),
                        )

                    # gate: v = tb - ta*ts ; t = sigmoid(v) ; nh = ff1 + t*d
                    # (only one PSUM input allowed per DVE op). psB is evicted
                    # to SBUF on ACT (hidden behind t1/v) so t3's single ACT
                    # wait covers both the sigmoid and [ff1|d].
                    t1 = gatep.tile([BL, U], f32, tag="t1")
                    nc.vector.tensor_scalar_mul(t1, psA[:, 0:U], ntss[:, st : st + 1])
                    v = gatep.tile([BL, U], f32, tag="v")
                    nc.vector.tensor_add(v, t1, psA[:, U : 2 * U])
                    fB = fbp.tile([BL, 2 * U], f32, tag="fb")
                    cb = nc.scalar.copy(fB, psB)
                    add_dep_helper(cb.ins, prev_act.ins, False, "act chain")
                    prev_act = cb
                    sg = gatep.tile([BL, U], f32, tag="sg")
                    sgi = nc.scalar.activation(sg, v, AF.Sigmoid)
                    add_dep_helper(sgi.ins, prev_act.ins, False, "act chain")
                    prev_act = sgi
                    t3 = gatep.tile([BL, U], f32, tag="t3")
                    nc.vector.tensor_mul(t3, sg, fB[:, U : 2 * U])
                    nh = nhp.tile([BL, U], f32, tag="nh")
                    nc.vector.tensor_add(nh, t3, fB[:, 0:U])
                    # fp16 downcast into the output staging tile (DVE copy)
                    nc.vector.tensor_copy(
                        ostage16[:, s * U : (s + 1) * U], nh
                    )

                    # hT for the next step: 4x 32x32 DVE transposes
                    hT = htp.tile([U, BL], f32, tag="ht")
                    for j in range(4):
                        nc.vector.transpose(
                            hT[32 * j : 32 * (j + 1), :],
                            nh[:, 32 * j : 32 * (j + 1)],
                        )
                    cur_hT = hT

                nc.gpsimd.dma_start(out=out_d[:, s0 : s0 + CHUNK, :], in_=ostage16)

            # final hidden state (transposed) for segment chaining
            nc.gpsimd.dma_start(out=hTo_d[:, :], in_=cur_hT)

    _drop_stale_self_waits(nc, mybir)
    return nc


def _drop_stale_self_waits(nc, mybir, margin=8):
    """Compute instructions have a single usable wait slot (the engine-sem
    update takes the other).  Tile emits same-engine/same-lane waits for
    slot reuse even when the producer is far back; on an in-order engine or
    FIFO DMA lane those are redundant.  Drop self waits on instructions
    carrying >1 wait: engine-sem waits when >= `margin` instructions stale,
    own-DMA-lane waits always (the lane is FIFO)."""
    eng_prefix = {
        mybir.EngineType.PE: "PE",
        mybir.EngineType.DVE: "DVE",
        mybir.EngineType.Activation: "Activation",
        mybir.EngineType.Pool: "Pool",
        mybir.EngineType.SP: "SP",
    }
    tick = {}
    eng_ic = {}  # engine -> instruction count so far
    reach = {}  # sem name -> list of (value, engine_instr_idx) in order
    for fn in nc.m.functions:
        for blk in fn.blocks:
            for i in blk.instructions:
                si = i.sync_info
                if si is None:
                    continue
                eng = getattr(i, "engine", None)
                pfx = eng_prefix.get(eng)
                my_ic = eng_ic.get(eng, 0)
                upd_sems = {u.ant_name for u in si.on_update}
                if len(si.on_wait) > 1:
                    is_dma = type(i).__name__ == "InstDMACopy"
                    kept = []
                    for w in si.on_wait:
                        n = w.ant_name
                        if pfx and n.startswith(pfx + "_"):
                            # same-engine self-wait: redundant whenever the
                            # producing instruction precedes this one on the
                            # same in-order engine (Tile itself relies on
                            # program order for all same-engine hazards)
                            hist = reach.get(n, [])
                            prod_ic = None
                            for v, ic in reversed(hist):
                                if v >= w.wait_value:
                                    prod_ic = ic
                                else:
                                    break
                            if prod_ic is not None and prod_ic <= my_ic:
                                continue  # program-order-satisfied self-wait
                        if (
                            is_dma
                            and n in upd_sems
                            and ("DMASW" in n or "DMAHW" in n)
                            and tick.get(n, 0) >= w.wait_value
                        ):
                            continue  # own-lane FIFO wait
                        kept.append(w)
                    if len(kept) != len(si.on_wait):
                        si.on_wait = kept
                for u in si.on_update:
                    v = tick.get(u.ant_name, 0) + u.update_value
                    tick[u.ant_name] = v
                    reach.setdefault(u.ant_name, []).append((v, my_ic))
                eng_ic[eng] = my_ic + 1
    _split_multiwait_drains(nc, mybir)


def _split_multiwait_drains(nc, mybir):
    """The kernel-tail Drain waits on every engine/DMA-lane sem, but its
    struct has a single wait slot.  Split: inject one single-wait Drain per
    extra wait immediately before it on the same engine."""
    for fn in nc.m.functions:
        for blk in fn.blocks:
            insts = blk.instructions
            out = []
            changed = False
            for i in insts:
                si = i.sync_info
                if type(i).__name__ == "InstDrain" and si and len(si.on_wait) > 1:
                    waits = list(si.on_wait)
                    for k, w in enumerate(waits[:-1]):
                        d = mybir.InstDrain(name=f"{i.name}-w{k}", ins=[], outs=[])
                        d.engine = i.engine
                        d.sync_info = mybir.SyncInfo(on_wait=[w], on_update=[])
                        out.append(d)
                    si.on_wait = [waits[-1]]
                    changed = True
                out.append(i)
            if changed:
                blk.instructions = out


def _prep_weights(W0, b0, W1, b1, W2, b2, Wa, ba, Wb, bb):
    W0 = np.asarray(W0, np.float32)
    W0x = W0[:C] / 100.0
    W0h = np.ascontiguousarray(W0[C:])  # [U, H]
    b0p = np.asarray(b0, np.float32) - 0.65 * W0[:C].sum(axis=0)
    W0aug = np.concatenate([W0x, b0p[None, :]], axis=0)  # [C+1, H]
    a = np.float32(1.7159)
    Wcat = np.concatenate([a * W1, a * (W2 - W1), a * Wa, a * Wb], axis=1)  # [H, 4U]
    bcat = np.concatenate([b1, b2 - b1, ba, bb]).astype(np.float32)  # [4U]
    return (
        W0aug.astype(np.float32),
        W0h.astype(np.float32),
        Wcat.astype(np.float32),
        bcat,
    )


def _make_blob(weights):
    W0aug, W0h, Wcat, bcat = weights
    blob = np.zeros((128, BLOB_COLS), np.float32)
    blob[:, _C_W0H : _C_W0H + H] = W0h
    for k2 in range(2):
        blob[:, _C_WCAT + k2 * 4 * U : _C_WCAT + (k2 + 1) * 4 * U] = Wcat[
            k2 * 128 : (k2 + 1) * 128, :
        ]
    blob[0, _C_BC : _C_BC + 4 * U] = bcat
    blob[0, _C_ONES : _C_ONES + BL] = 1.0
    return blob


class _Res:
    exec_time_ns = None
    mean_exec_time_ns = None
    instructions_and_trace = None
    profile_json = None


_CACHE = {}
_CACHE_LOCK = threading.Lock()


def _get_rt(s_seg):
    """Build (once) the Bass program + jitted sharded callable for a segment
    length, plus cached device-resident output placeholders."""
    key = ("rt", s_seg)
    with _CACHE_LOCK:
        if key in _CACHE:
            return _CACHE[key]
    import jax
    from jax.sharding import Mesh, PartitionSpec, NamedSharding
    from jax.experimental.shard_map import shard_map
    from concourse import mybir
    from concourse.bass2jax import (
        _bass_exec_p,
        install_neuronx_cc_hook,
        partition_id_tensor,
    )

    install_neuronx_cc_hook()
    nc = _build_nc(s_seg)

    in_names, out_names, out_avals = [], [], []
    for alloc in nc.m.functions[0].allocations:
        if not isinstance(alloc, mybir.MemoryLocationSet):
            continue
        name = alloc.memorylocations[0].name
        if alloc.kind == "ExternalInput":
            in_names.append(name)
        elif alloc.kind == "ExternalOutput":
            out_names.append(name)
            out_avals.append(
                jax.core.ShapedArray(
                    tuple(alloc.tensor_shape), mybir.dt.np(alloc.dtype)
                )
            )
    partition_name = nc.partition_id_tensor.name if nc.partition_id_tensor else None
    if partition_name is not None:
        in_names.remove(partition_name)
    all_in = in_names + out_names

    def _body(*args):
        operands = list(args)
        if partition_name is not None:
            operands.append(partition_id_tensor())
        outs = _bass_exec_p.bind(
            *operands,
            out_avals=tuple(out_avals),
            in_names=tuple(all_in + ([partition_name] if partition_name else [])),
            out_names=tuple(out_names),
            lowering_input_output_aliases=(),
            sim_require_finite=True,
            sim_require_nnan=True,
            nc=nc,
        )
        return tuple(outs)

    devices = jax.devices()[:NCORES]
    mesh = Mesh(np.asarray(devices), ("core",))
    P = PartitionSpec
    jitted = jax.jit(
        shard_map(
            _body,
            mesh=mesh,
            in_specs=(P("core"),) * len(all_in),
            out_specs=(P("core"),) * len(out_names),
            check_rep=False,
        ),
        keep_unused=True,
    )
    sh = NamedSharding(mesh, P("core"))
    ph_out = jax.device_put(np.zeros((NCORES * BL, s_seg, U), np.float16), sh)
    ph_hT = jax.device_put(np.zeros((NCORES * U, BL), np.float32), sh)
    rt = dict(
        nc=nc, jitted=jitted, sh=sh, in_names=in_names, out_names=out_names,
        ph_out=ph_out, ph_hT=ph_hT,
    )
    with _CACHE_LOCK:
        _CACHE[key] = rt
    return rt


def _weights_dev(weights, sh):
    """Device-resident replicated blob, cached per weights object."""
    import jax

    key = ("wdev", id(weights))
    with _CACHE_LOCK:
        hit = _CACHE.get(key)
    if hit is not None:
        return hit
    blob = _make_blob(weights)  # [128, BLOB_COLS] f32
    blob_g = np.broadcast_to(blob, (NCORES, *blob.shape)).reshape(
        NCORES * 128, BLOB_COLS
    )
    blob_dev = jax.device_put(np.ascontiguousarray(blob_g), sh)
    with _CACHE_LOCK:
        _CACHE[key] = blob_dev
    return blob_dev


def _w0a_dev(weights, xmax, sh):
    """Per-call fp16 W0aug with the int8 dequant scale folded into the
    x rows (33KB upload)."""
    import jax

    w0a = weights[0].copy()  # [C+1, H] f32
    w0a[:C] *= np.float32(xmax / 127.0)
    w0a_g = np.broadcast_to(w0a.astype(np.float16), (NCORES, C + 1, H)).reshape(
        NCORES * (C + 1), H
    )
    return jax.device_put(np.ascontiguousarray(w0a_g), sh)


def _prep_x_seg(xq, s0, s_seg):
    """[B, S, C] int8 -> per-core-concat [8*(C+1), s_seg, BL] int8."""
    xg = np.empty((NCORES * (C + 1), s_seg, BL), np.int8)
    for c in range(NCORES):
        blk = xg[c * (C + 1) : (c + 1) * (C + 1)]
        blk[:C] = xq[c * BL : (c + 1) * BL, s0 : s0 + s_seg, :].transpose(2, 1, 0)
        blk[C] = 1  # ones plane carries the (unscaled) bias row
    return xg


def _fetch_shard(full, s0, s_seg, shard):
    c = shard.index[0].start // BL
    arr = np.asarray(shard.data)  # [BL, s_seg, U] f16 (blocks until ready)
    full[c * BL : (c + 1) * BL, s0 : s0 + s_seg] = arr  # f16 -> f32 cast


def run(x_codes, h0, timespans, weights, s_total=S, trace=False):
    import jax

    s_seg = min(SEG, s_total)
    assert s_total % s_seg == 0 and s_seg % CHUNK == 0
    nseg = s_total // s_seg
    rt = _get_rt(s_seg)
    blob_dev = _weights_dev(weights, rt["sh"])

    xf = np.asarray(x_codes, np.float32)[:, :s_total]
    xmax = float(max(xf.max(), -float(xf.min()), 1e-30))
    w0a_dev = _w0a_dev(weights, xmax, rt["sh"])
    xq = np.clip(np.rint(xf * np.float32(127.0 / xmax)), -127, 127).astype(np.int8)
    nts = -np.asarray(timespans, np.float32)[:, :s_total]
    h0 = np.asarray(h0, np.float32)
    h0T_g = np.ascontiguousarray(
        h0.reshape(NCORES, BL, U).transpose(0, 2, 1).reshape(NCORES * U, BL)
    )
    h_cur = jax.device_put(h0T_g, rt["sh"])

    by_name_static = {"blob": blob_dev, "w0a": w0a_dev}
    full = np.empty((B, s_total, U), np.float32)
    futs = []
    with ThreadPoolExecutor(max_workers=8) as ex:
        for si in range(nseg):
            s0 = si * s_seg
            xg = _prep_x_seg(xq, s0, s_seg)
            ng = np.ascontiguousarray(nts[:, s0 : s0 + s_seg])
            x_dev = jax.device_put(xg, rt["sh"])
            n_dev = jax.device_put(ng, rt["sh"])
            by_name = {
                **by_name_static,
                "xT": x_dev, "nts": n_dev, "h0T": h_cur,
                "out": rt["ph_out"], "hTout": rt["ph_hT"],
            }
            args = [by_name[n] for n in rt["in_names"] + rt["out_names"]]
            res = rt["jitted"](*args)
            outs = dict(zip(rt["out_names"], res))
            h_cur = outs["hTout"]
            for shard in outs["out"].addressable_shards:
                futs.append(ex.submit(_fetch_shard, full, s0, s_seg, shard))
        for f in futs:
            f.result()
    return full, _Res()


def kernel(x_codes, h0, timespans, W0, b0, W1, b1, W2, b2, Wa, ba, Wb, bb):
    weights = _prep_weights(W0, b0, W1, b1, W2, b2, Wa, ba, Wb, bb)
    full, _ = run(
        np.asarray(x_codes, np.float32),
        np.asarray(h0, np.float32),
        np.asarray(timespans, np.float32),
        weights,
        S,
    )
    return full.astype(np.float32)


# revision 11
# speedup vs baseline: 1.1953x; 1.0628x over previous
"""CfC RNN scan kernel for Trainium2 (8 NeuronCores, data-parallel over batch).

Math (per step, from the reference):
    f   = 1.7159 * tanh(0.666 * (concat(x_s, h) @ W0 + b0))     x_s = (x-65)/100
    ff1 = f @ W1 + b1 ;  ff2 = f @ W2 + b2
    ta  = f @ Wa + ba ;  tb  = f @ Wb + bb
    t   = sigmoid(tb - ta * ts)
    h'  = ff1 + t * (ff2 - ff1)

Folding done on the host:
  - input scale/shift folded into W0x, b0:  xterm = x @ (W0x/100) + (b0 - .65*W0x.sum(0))
  - 1.7159 folded into the head weights; heads consume g = tanh(0.666*z) directly
  - d = ff2-ff1 computed via Wd = W2-W1, bd = b2-b1
  - head weights concatenated: Wcat = [W1' | Wd' | Wa' | Wb'] (256 x 512)

End-to-end: the wall-clock here is dominated by the axon tunnel (~60MB/s up,
~35MB/s down), so the runner minimizes wire bytes and overlaps transfers:
  - x ships as fp16 [C+1, S, BL] (half the bytes); the x-term matmuls run in
    fp16 against an fp16 copy of W0aug (error ~1e-5 on z, way under tol).
  - the output ships as fp16 (one ACT downcast per 32-step chunk on-device),
    upcast to f32 on the host. Per-element rel err ~5e-4 vs 2e-2 tolerance.
  - the scan is cut into SEG-step segments chained through a device-resident
    hT state tensor; uploads of segment i+1 and downloads of segment i overlap
    via jax async dispatch + background fetch threads.
  - no 256MB zero-init upload: output placeholder operands are dead (the NEFF
    output binds to the custom-call result buffer), one cached dummy is reused.
  - the jitted executable, device-resident weights, and placeholders are
    cached across calls, so a steady-state call is pure transfer + execute.

On-chip structure (per core, B_local=32):
  - x is fed pre-transposed as xT [C+1, S, BL] fp16 (row C = ones so b0 rides
    the matmul); per 32-step chunk one DMA stages it; per step a matmul pair
    computes the x-dependent backbone term straight into PSUM; the recurrent
    f32 matmul accumulates on top (no eviction/preload).
  - Persistent constants live in a single f32 "blob" (W0h, Wcat, bcat, ones)
    plus a small fp16 W0aug tensor and the f32 h0T state, one DMA each: the
    HW Matmult instruction tolerates a single semaphore wait, so three chained
    1x1 warm-up matmuls absorb the three DMA waits before any real matmul.
  - scan step: hT [128,32] -> MM1 accumulate -> ACT tanh [128,2,32] -> g;
    heads use g as the (P=32) stationary operand: psA=[ta|tb], psB=[ff1|d] in
    separate PSUM banks; per-bank K=1 ones-row matmuls add the biases
    (h-independent, off the critical path).
  - gate: DVE tensor_scalar (ta*-ts, PSUM->SBUF), DVE add (+tb), ACT sigmoid,
    DVE mul (*d), DVE add (+ff1) written into the f32 output staging tile; 4
    DVE 32x32 transposes produce hT for the next step.  At chunk end one ACT
    copy downcasts the staging tile to fp16 for the out DMA.
"""

import sys
import threading
from concurrent.futures import ThreadPoolExecutor

import numpy as np

for _p in ("/opt/trn_rl_repo",):
    if _p not in sys.path:
        sys.path.insert(0, _p)

B, S, C, U, H = 256, 2048, 64, 128, 256
NCORES = 8
BL = B // NCORES  # 32
CHUNK = 32
TS_SUPER = 256  # steps per timespan staging DMA
SEG = 512  # steps per device program (pipeline granularity)

# blob column layout (128 partitions x BLOB_COLS fp32)
_C_W0H = 0            # [128, 256]
_C_WCAT = 256         # [128, 1024] = 2 K-tiles x 512
_C_BC = 1280          # [1, 512] bcat (rows 1..127 stay zero -> zrow)
_C_ONES = 1792        # [1, 32] ones
BLOB_COLS = 1824


def _build_nc(s_total: int):
    import concourse.bass as bass
    import concourse.tile as tile
    from concourse import mybir
    from concourse.tile_rust import add_dep_helper
    import concourse.tile_sem_assignment as _tsa

    # All DMAs go through gpsimd/SWDGE; cap the SWDGE sem count so the
    # kernel-tail Drain's per-queue waits fit its struct's wait slots.
    _tsa.NUM_SWDGE_GLOBAL_SEMS = 2

    f32 = mybir.dt.float32
    f16 = mybir.dt.float16
    AF = mybir.ActivationFunctionType
    nchunk = s_total // CHUNK
    ts_super = min(TS_SUPER, s_total)

    nc = bass.Bass("TRN2")
    i8 = mybir.dt.int8
    xT_d = nc.dram_tensor("xT", [C + 1, s_total, BL], i8, kind="ExternalInput")
    nts_d = nc.dram_tensor("nts", [BL, s_total], f32, kind="ExternalInput")
    blob_d = nc.dram_tensor("blob", [128, BLOB_COLS], f32, kind="ExternalInput")
    w0a_d = nc.dram_tensor("w0a", [C + 1, H], f16, kind="ExternalInput")
    h0T_d = nc.dram_tensor("h0T", [U, BL], f32, kind="ExternalInput")
    out_d = nc.dram_tensor("out", [BL, s_total, U], f16, kind="ExternalOutput")
    hTo_d = nc.dram_tensor("hTout", [U, BL], f32, kind="ExternalOutput")

    with tile.TileContext(nc) as tc:
        with (
            tc.tile_pool(name="singles", bufs=1) as singles,
            tc.tile_pool(name="xstage", bufs=2) as xstage,
            tc.tile_pool(name="xfst", bufs=2) as xfstage,
            tc.tile_pool(name="tsstage", bufs=2) as tsstage,
            tc.tile_pool(name="outstage16", bufs=2) as outstage16,
            tc.tile_pool(name="ft", bufs=6) as ftp,
            tc.tile_pool(name="fb", bufs=6) as fbp,
            tc.tile_pool(name="gate", bufs=6) as gatep,
            tc.tile_pool(name="nh", bufs=3) as nhp,
            tc.tile_pool(name="ht", bufs=2) as htp,
            tc.tile_pool(name="psf", bufs=3, space="PSUM") as psfp,
            tc.tile_pool(name="psbnd", bufs=1, space="PSUM") as psbndp,
            tc.tile_pool(name="psa", bufs=2, space="PSUM") as psap,
            tc.tile_pool(name="psb", bufs=2, space="PSUM") as psbp,
        ):
            sb_blob = singles.tile([128, BLOB_COLS], f32, tag="blob")
            nc.gpsimd.dma_start(out=sb_blob, in_=blob_d[:, :])
            sb_w0a = singles.tile([C + 1, H], f16, tag="w0a")
            nc.gpsimd.dma_start(out=sb_w0a, in_=w0a_d[:, :])
            sb_h0T = singles.tile([U, BL], f32, tag="h0T")
            nc.gpsimd.dma_start(out=sb_h0T, in_=h0T_d[:, :])

            sb_W0h = sb_blob[:, _C_W0H : _C_W0H + H]
            sb_scr = singles.tile([1, 16], f32, tag="scratch")
            # a zero row of the blob: row 64 of the bcat column range (only
            # row 0 holds data there); base partition must be 0/32/64
            sb_zrow = sb_blob[64:65, _C_BC : _C_BC + 256]
            sb_bcat = sb_blob[0:1, _C_BC : _C_BC + 4 * U]
            sb_ones = sb_blob[0:1, _C_ONES : _C_ONES + BL]

            def wcat(k2, lo, hi):
                base = _C_WCAT + k2 * 4 * U
                return sb_blob[:, base + lo : base + hi]

            # warm-up: three 1x1 matmuls so PE observes each input DMA's
            # semaphore before any real matmul (Matmult carries at most one
            # sync wait); PE is in-order so they need no inter-deps.
            ps_w = psap.tile([BL, 2 * U], f32, tag="psa")
            nc.tensor.matmul(
                ps_w[0:1, 0:1], sb_blob[0:1, 0:1], sb_blob[0:1, 0:1],
                start=True, stop=True,
            )
            nc.tensor.matmul(
                ps_w[0:1, 0:1], sb_w0a[0:1, 0:1], sb_w0a[0:1, 0:1],
                start=True, stop=True,
            )
            nc.tensor.matmul(
                ps_w[0:1, 0:1], sb_h0T[0:1, 0:1], sb_h0T[0:1, 0:1],
                start=True, stop=True,
            )

            cur_hT = sb_h0T
            prev_pe = None  # last PE instruction of the previous step
            prev_act = None  # nosync chain pinning the ACT instruction order

            for ci in range(nchunk):
                s0 = ci * CHUNK
                xTa8 = xstage.tile([C + 1, CHUNK * BL], i8, tag="xta8")
                nc.gpsimd.dma_start(out=xTa8, in_=xT_d[:, s0 : s0 + CHUNK, :])
                xTa = xfstage.tile([C + 1, CHUNK * BL], f16, tag="xta")
                # ACT toucher: absorb the WAR vs this buffer's PE readers two
                # chunks back, so the convert below carries only the DMA wait
                xt_t = nc.scalar.copy(xTa[0:1, 0:1], sb_blob[0:1, 0:1])
                if prev_act is not None:
                    add_dep_helper(xt_t.ins, prev_act.ins, False, "act chain")
                prev_act = xt_t
                # dequantize int8 -> fp16 (scale is folded into w0a host-side)
                xt_c = nc.scalar.copy(xTa, xTa8)
                add_dep_helper(xt_c.ins, prev_act.ins, False, "act chain")
                prev_act = xt_c
                if s0 % ts_super == 0:
                    ntss = tsstage.tile([BL, ts_super], f32, tag="ntss")
                    nc.gpsimd.dma_start(out=ntss, in_=nts_d[:, s0 : s0 + ts_super])
                    # DVE toucher: absorb the ntss DMA wait on DVE once, so
                    # per-step tensor_scalar ops don't carry a second wait
                    sci = s0 // ts_super
                    nc.vector.tensor_copy(
                        sb_scr[0:1, sci : sci + 1], ntss[0:1, 0:1]
                    )

                ostage16 = outstage16.tile([BL, CHUNK * U], f16, tag="ostage16")
                # DVE toucher: absorb the WAR on the previous out-DMA of this
                # staging buffer so the per-step fp16 copies have only one wait
                nc.vector.memset(ostage16[0:1, 0:1], 0.0)

                for s in range(CHUNK):
                    st = (s0 + s) % ts_super  # index into ntss
                    # backbone: z = x-term + W0h.T @ hT, one accumulation group
                    # per m-tile (the x-term matmul is h-independent and runs
                    # ahead; same-group accumulation avoids extra PE waits)
                    # chunk-boundary step uses a dedicated psum tile: its
                    # slot-reuse WAW wait is then chunk-distant (dominated),
                    # leaving room for the xTa DMA wait (1-wait limit)
                    if s == 0:
                        ps_f = psbndp.tile([128, 2, BL], f32, tag="psbnd")
                    else:
                        ps_f = psfp.tile([128, 2, BL], f32, tag="psf")
                    # start=True clears the ENTIRE psum bank, so the two
                    # m-tiles (sharing one bank) must not each lead their own
                    # group: one K=1 zero-matmul clears/claims the whole
                    # region, everything else accumulates.
                    clr = nc.tensor.matmul(
                        ps_f,
                        sb_zrow[:, 0:128],
                        sb_zrow[:, 0 : 2 * BL],
                        start=True,
                        stop=False,
                        skip_group_check=True,
                    )
                    if prev_pe is not None:
                        add_dep_helper(clr.ins, prev_pe.ins, False, "clr after heads")
                    for m in range(2):
                        nc.tensor.matmul(
                            ps_f[:, m, :],
                            sb_w0a[:, m * 128 : (m + 1) * 128],
                            xTa[:, s * BL : (s + 1) * BL],
                            start=False,
                            stop=False,
                            skip_group_check=True,
                        )
                    mm1_last = None
                    for m in range(2):
                        mm1_last = nc.tensor.matmul(
                            ps_f[:, m, :],
                            sb_W0h[:, m * 128 : (m + 1) * 128],
                            cur_hT,
                            start=False,
                            stop=True,
                            skip_group_check=True,
                        )
                    # g = tanh(0.666 * z), both H-tiles in one ACT op
                    fT = ftp.tile([128, 2, BL], f32, tag="ft")
                    th = nc.scalar.activation(fT, ps_f, AF.Tanh, scale=0.666)
                    if prev_act is not None:
                        # nosync chain: fixes the ACT stream order so slot
                        # reuse stays outside the queue window and no ACT
                        # self-waits are emitted (Activation has 1 wait slot)
                        add_dep_helper(th.ins, prev_act.ins, False, "act chain")
                    prev_act = th

                    # heads: psA = [ta | tb], psB = [ff1 | d] (separate banks)
                    psA = psap.tile([BL, 2 * U], f32, tag="psa")
                    psB = psbp.tile([BL, 2 * U], f32, tag="psb")
                    # order-only dep: keep the bias matmuls behind this
                    # step's MM1 so their psum-WAR wait is dominated by MM1's
                    # DVE wait (Matmult tolerates only one sync wait)
                    bmA = nc.tensor.matmul(
                        psA, sb_ones, sb_bcat[:, 2 * U : 4 * U], start=True, stop=False
                    )
                    bmB = nc.tensor.matmul(
                        psB, sb_ones, sb_bcat[:, 0 : 2 * U], start=True, stop=False
                    )
                    add_dep_helper(bmA.ins, mm1_last.ins, False, "bias after MM1")
                    add_dep_helper(bmB.ins, mm1_last.ins, False, "bias after MM1")
                    for k2 in range(2):
                        nc.tensor.matmul(
                            psA,
                            fT[:, k2, :],
                            wcat(k2, 2 * U, 4 * U),
                            start=False,
                            stop=(k2 == 1),
                        )
                    for k2 in range(2):
                        prev_pe = nc.tensor.matmul(
                            psB,
                            fT[:, k2, :],
                            wcat(k2, 0, 2 * U),
                            start=False,
                            stop=(k2 == 1),
                        )

                    # gate: v = tb - ta*ts ; t = sigmoid(v) ; nh = ff1 + t*d
                    # (only one PSUM input allowed per DVE op). psB is evicted
                    # to SBUF on ACT (hidden behind t1/v) so t3's single ACT
                    # wait covers both the sigmoid and [ff1|d].
                    t1 = gatep.tile([BL, U], f32, tag="t1")
                    nc.vector.tensor_scalar_mul(t1, psA[:, 0:U], ntss[:, st : st + 1])
                    v = gatep.tile([BL, U], f32, tag="v")
                    nc.vector.tensor_add(v, t1, psA[:, U : 2 * U])
                    fB = fbp.tile([BL, 2 * U], f32, tag="fb")
                    cb = nc.scalar.copy(fB, psB)
                    add_dep_helper(cb.ins, prev_act.ins, False, "act chain")
                    prev_act = cb
                    sg = gatep.tile([BL, U], f32, tag="sg")
                    sgi = nc.scalar.activation(sg, v, AF.Sigmoid)
                    add_dep_helper(sgi.ins, prev_act.ins, False, "act chain")
                    prev_act = sgi
                    t3 = gatep.tile([BL, U], f32, tag="t3")
                    nc.vector.tensor_mul(t3, sg, fB[:, U : 2 * U])
                    nh = nhp.tile([BL, U], f32, tag="nh")
                    nc.vector.tensor_add(nh, t3, fB[:, 0:U])
                    # fp16 downcast into the output staging tile (DVE copy)
                    nc.vector.tensor_copy(
                        ostage16[:, s * U : (s + 1) * U], nh
                    )

                    # hT for the next step: 4x 32x32 DVE transposes
                    hT = htp.tile([U, BL], f32, tag="ht")
                    for j in range(4):
                        nc.vector.transpose(
                            hT[32 * j : 32 * (j + 1), :],
                            nh[:, 32 * j : 32 * (j + 1)],
                        )
                    cur_hT = hT

                nc.gpsimd.dma_start(out=out_d[:, s0 : s0 + CHUNK, :], in_=ostage16)

            # final hidden state (transposed) for segment chaining
            nc.gpsimd.dma_start(out=hTo_d[:, :], in_=cur_hT)

    _drop_stale_self_waits(nc, mybir)
    return nc


def _drop_stale_self_waits(nc, mybir, margin=8):
    """Compute instructions have a single usable wait slot (the engine-sem
    update takes the other).  Tile emits same-engine/same-lane waits for
    slot reuse even when the producer is far back; on an in-order engine or
    FIFO DMA lane those are redundant.  Drop self waits on instructions
    carrying >1 wait: engine-sem waits when >= `margin` instructions stale,
    own-DMA-lane waits always (the lane is FIFO)."""
    eng_prefix = {
        mybir.EngineType.PE: "PE",
        mybir.EngineType.DVE: "DVE",
        mybir.EngineType.Activation: "Activation",
        mybir.EngineType.Pool: "Pool",
        mybir.EngineType.SP: "SP",
    }
    tick = {}
    eng_ic = {}  # engine -> instruction count so far
    reach = {}  # sem name -> list of (value, engine_instr_idx) in order
    for fn in nc.m.functions:
        for blk in fn.blocks:
            for i in blk.instructions:
                si = i.sync_info
                if si is None:
                    continue
                eng = getattr(i, "engine", None)
                pfx = eng_prefix.get(eng)
                my_ic = eng_ic.get(eng, 0)
                upd_sems = {u.ant_name for u in si.on_update}
                if len(si.on_wait) > 1:
                    is_dma = type(i).__name__ == "InstDMACopy"
                    kept = []
                    for w in si.on_wait:
                        n = w.ant_name
                        if pfx and n.startswith(pfx + "_"):
                            # same-engine self-wait: redundant whenever the
                            # producing instruction precedes this one on the
                            # same in-order engine (Tile itself relies on
                            # program order for all same-engine hazards)
                            hist = reach.get(n, [])
                            prod_ic = None
                            for v, ic in reversed(hist):
                                if v >= w.wait_value:
                                    prod_ic = ic
                                else:
                                    break
                            if prod_ic is not None and prod_ic <= my_ic:
                                continue  # program-order-satisfied self-wait
                        if (
                            is_dma
                            and n in upd_sems
                            and ("DMASW" in n or "DMAHW" in n)
                            and tick.get(n, 0) >= w.wait_value
                        ):
                            continue  # own-lane FIFO wait
                        kept.append(w)
                    if len(kept) != len(si.on_wait):
                        si.on_wait = kept
                for u in si.on_update:
                    v = tick.get(u.ant_name, 0) + u.update_value
                    tick[u.ant_name] = v
                    reach.setdefault(u.ant_name, []).append((v, my_ic))
                eng_ic[eng] = my_ic + 1
    _split_multiwait_drains(nc, mybir)


def _split_multiwait_drains(nc, mybir):
    """The kernel-tail Drain waits on every engine/DMA-lane sem, but its
    struct has a single wait slot.  Split: inject one single-wait Drain per
    extra wait immediately before it on the same engine."""
    for fn in nc.m.functions:
        for blk in fn.blocks:
            insts = blk.instructions
            out = []
            changed = False
            for i in insts:
                si = i.sync_info
                if type(i).__name__ == "InstDrain" and si and len(si.on_wait) > 1:
                    waits = list(si.on_wait)
                    for k, w in enumerate(waits[:-1]):
                        d = mybir.InstDrain(name=f"{i.name}-w{k}", ins=[], outs=[])
                        d.engine = i.engine
                        d.sync_info = mybir.SyncInfo(on_wait=[w], on_update=[])
                        out.append(d)
                    si.on_wait = [waits[-1]]
                    changed = True
                out.append(i)
            if changed:
                blk.instructions = out


def _prep_weights(W0, b0, W1, b1, W2, b2, Wa, ba, Wb, bb):
    W0 = np.asarray(W0, np.float32)
    W0x = W0[:C] / 100.0
    W0h = np.ascontiguousarray(W0[C:])  # [U, H]
    b0p = np.asarray(b0, np.float32) - 0.65 * W0[:C].sum(axis=0)
    W0aug = np.concatenate([W0x, b0p[None, :]], axis=0)  # [C+1, H]
    a = np.float32(1.7159)
    Wcat = np.concatenate([a * W1, a * (W2 - W1), a * Wa, a * Wb], axis=1)  # [H, 4U]
    bcat = np.concatenate([b1, b2 - b1, ba, bb]).astype(np.float32)  # [4U]
    return (
        W0aug.astype(np.float32),
        W0h.astype(np.float32),
        Wcat.astype(np.float32),
        bcat,
    )


def _make_blob(weights):
    W0aug, W0h, Wcat, bcat = weights
    blob = np.zeros((128, BLOB_COLS), np.float32)
    blob[:, _C_W0H : _C_W0H + H] = W0h
    for k2 in range(2):
        blob[:, _C_WCAT + k2 * 4 * U : _C_WCAT + (k2 + 1) * 4 * U] = Wcat[
            k2 * 128 : (k2 + 1) * 128, :
        ]
    blob[0, _C_BC : _C_BC + 4 * U] = bcat
    blob[0, _C_ONES : _C_ONES + BL] = 1.0
    return blob


class _Res:
    exec_time_ns = None
    mean_exec_time_ns = None
    instructions_and_trace = None
    profile_json = None


_CACHE = {}
_CACHE_LOCK = threading.Lock()


def _get_rt(s_seg):
    """Build (once) the Bass program + jitted sharded callable for a segment
    length, plus cached device-resident output placeholders."""
    key = ("rt", s_seg)
    with _CACHE_LOCK:
        if key in _CACHE:
            return _CACHE[key]
    import jax
    from jax.sharding import Mesh, PartitionSpec, NamedSharding
    from jax.experimental.shard_map import shard_map
    from concourse import mybir
    from concourse.bass2jax import (
        _bass_exec_p,
        install_neuronx_cc_hook,
        partition_id_tensor,
    )

    install_neuronx_cc_hook()
    nc = _build_nc(s_seg)

    in_names, out_names, out_avals = [], [], []
    for alloc in nc.m.functions[0].allocations:
        if not isinstance(alloc, mybir.MemoryLocationSet):
            continue
        name = alloc.memorylocations[0].name
        if alloc.kind == "ExternalInput":
            in_names.append(name)
        elif alloc.kind == "ExternalOutput":
            out_names.append(name)
            out_avals.append(
                jax.core.ShapedArray(
                    tuple(alloc.tensor_shape), mybir.dt.np(alloc.dtype)
                )
            )
    partition_name = nc.partition_id_tensor.name if nc.partition_id_tensor else None
    if partition_name is not None:
        in_names.remove(partition_name)
    all_in = in_names + out_names

    def _body(*args):
        operands = list(args)
        if partition_name is not None:
            operands.append(partition_id_tensor())
        outs = _bass_exec_p.bind(
            *operands,
            out_avals=tuple(out_avals),
            in_names=tuple(all_in + ([partition_name] if partition_name else [])),
            out_names=tuple(out_names),
            lowering_input_output_aliases=(),
            sim_require_finite=True,
            sim_require_nnan=True,
            nc=nc,
        )
        return tuple(outs)

    devices = jax.devices()[:NCORES]
    mesh = Mesh(np.asarray(devices), ("core",))
    P = PartitionSpec
    jitted = jax.jit(
        shard_map(
            _body,
            mesh=mesh,
            in_specs=(P("core"),) * len(all_in),
            out_specs=(P("core"),) * len(out_names),
            check_rep=False,
        ),
        keep_unused=True,
    )
    sh = NamedSharding(mesh, P("core"))
    ph_out = jax.device_put(np.zeros((NCORES * BL, s_seg, U), np.float16), sh)
    ph_hT = jax.device_put(np.zeros((NCORES * U, BL), np.float32), sh)
    rt = dict(
        nc=nc, jitted=jitted, sh=sh, in_names=in_names, out_names=out_names,
        ph_out=ph_out, ph_hT=ph_hT,
    )
    with _CACHE_LOCK:
        _CACHE[key] = rt
    return rt


def _weights_dev(weights, sh):
    """Device-resident replicated blob, cached per weights object."""
    import jax

    key = ("wdev", id(weights))
    with _CACHE_LOCK:
        hit = _CACHE.get(key)
    if hit is not None:
        return hit
    blob = _make_blob(weights)  # [128, BLOB_COLS] f32
    blob_g = np.broadcast_to(blob, (NCORES, *blob.shape)).reshape(
        NCORES * 128, BLOB_COLS
    )
    blob_dev = jax.device_put(np.ascontiguousarray(blob_g), sh)
    with _CACHE_LOCK:
        _CACHE[key] = blob_dev
    return blob_dev


def _w0a_dev(weights, xmax, sh):
    """Per-call fp16 W0aug with the int8 dequant scale folded into the
    x rows (33KB upload)."""
    import jax

    w0a = weights[0].copy()  # [C+1, H] f32
    w0a[:C] *= np.float32(xmax / 127.0)
    w0a_g = np.broadcast_to(w0a.astype(np.float16), (NCORES, C + 1, H)).reshape(
        NCORES * (C + 1), H
    )
    return jax.device_put(np.ascontiguousarray(w0a_g), sh)


def _prep_x_seg(xf, sc, s0, s_seg):
    """Quantize + transpose one segment: [B, S, C] f32 -> [8*(C+1), s_seg, BL]
    int8 (done per segment so it hides under the wire transfers)."""
    xg = np.empty((NCORES * (C + 1), s_seg, BL), np.int8)
    q = np.clip(np.rint(xf[:, s0 : s0 + s_seg, :] * sc), -127, 127).astype(np.int8)
    for c in range(NCORES):
        blk = xg[c * (C + 1) : (c + 1) * (C + 1)]
        blk[:C] = q[c * BL : (c + 1) * BL].transpose(2, 1, 0)
        blk[C] = 1  # ones plane carries the (unscaled) bias row
    return xg


def _fetch_shard(full, s0, s_seg, shard):
    c = shard.index[0].start // BL
    arr = np.asarray(shard.data)  # [BL, s_seg, U] f16 (blocks until ready)
    full[c * BL : (c + 1) * BL, s0 : s0 + s_seg] = arr  # f16 -> f32 cast


def run(x_codes, h0, timespans, weights, s_total=S, trace=False):
    import jax

    s_seg = min(SEG, s_total)
    assert s_total % s_seg == 0 and s_seg % CHUNK == 0
    nseg = s_total // s_seg
    rt = _get_rt(s_seg)
    blob_dev = _weights_dev(weights, rt["sh"])

    xf = np.asarray(x_codes, np.float32)[:, :s_total]
    xmax = float(max(xf.max(), -float(xf.min()), 1e-30))
    w0a_dev = _w0a_dev(weights, xmax, rt["sh"])
    sc = np.float32(127.0 / xmax)
    nts = -np.asarray(timespans, np.float32)[:, :s_total]
    h0 = np.asarray(h0, np.float32)
    h0T_g = np.ascontiguousarray(
        h0.reshape(NCORES, BL, U).transpose(0, 2, 1).reshape(NCORES * U, BL)
    )
    h_cur = jax.device_put(h0T_g, rt["sh"])

    by_name_static = {"blob": blob_dev, "w0a": w0a_dev}
    full = np.empty((B, s_total, U), np.float32)
    futs = []
    with ThreadPoolExecutor(max_workers=8) as ex:
        for si in range(nseg):
            s0 = si * s_seg
            xg = _prep_x_seg(xf, sc, s0, s_seg)
            ng = np.ascontiguousarray(nts[:, s0 : s0 + s_seg])
            x_dev = jax.device_put(xg, rt["sh"])
            n_dev = jax.device_put(ng, rt["sh"])
            by_name = {
                **by_name_static,
                "xT": x_dev, "nts": n_dev, "h0T": h_cur,
                "out": rt["ph_out"], "hTout": rt["ph_hT"],
            }
            args = [by_name[n] for n in rt["in_names"] + rt["out_names"]]
            res = rt["jitted"](*args)
            outs = dict(zip(rt["out_names"], res))
            h_cur = outs["hTout"]
            for shard in outs["out"].addressable_shards:
                futs.append(ex.submit(_fetch_shard, full, s0, s_seg, shard))
        for f in futs:
            f.result()
    return full, _Res()


def kernel(x_codes, h0, timespans, W0, b0, W1, b1, W2, b2, Wa, ba, Wb, bb):
    weights = _prep_weights(W0, b0, W1, b1, W2, b2, Wa, ba, Wb, bb)
    full, _ = run(
        np.asarray(x_codes, np.float32),
        np.asarray(h0, np.float32),
        np.asarray(timespans, np.float32),
        weights,
        S,
    )
    return full.astype(np.float32)


# revision 12
# speedup vs baseline: 1.2009x; 1.0047x over previous
"""CfC RNN scan kernel for Trainium2 (8 NeuronCores, data-parallel over batch).

Math (per step, from the reference):
    f   = 1.7159 * tanh(0.666 * (concat(x_s, h) @ W0 + b0))     x_s = (x-65)/100
    ff1 = f @ W1 + b1 ;  ff2 = f @ W2 + b2
    ta  = f @ Wa + ba ;  tb  = f @ Wb + bb
    t   = sigmoid(tb - ta * ts)
    h'  = ff1 + t * (ff2 - ff1)

Folding done on the host:
  - input scale/shift folded into W0x, b0:  xterm = x @ (W0x/100) + (b0 - .65*W0x.sum(0))
  - 1.7159 folded into the head weights; heads consume g = tanh(0.666*z) directly
  - d = ff2-ff1 computed via Wd = W2-W1, bd = b2-b1
  - head weights concatenated: Wcat = [W1' | Wd' | Wa' | Wb'] (256 x 512)

End-to-end: the wall-clock here is dominated by the axon tunnel (~60MB/s up,
~35MB/s down), so the runner minimizes wire bytes and overlaps transfers:
  - x ships as fp16 [C+1, S, BL] (half the bytes); the x-term matmuls run in
    fp16 against an fp16 copy of W0aug (error ~1e-5 on z, way under tol).
  - the output ships as fp16 (one ACT downcast per 32-step chunk on-device),
    upcast to f32 on the host. Per-element rel err ~5e-4 vs 2e-2 tolerance.
  - the scan is cut into SEG-step segments chained through a device-resident
    hT state tensor; uploads of segment i+1 and downloads of segment i overlap
    via jax async dispatch + background fetch threads.
  - no 256MB zero-init upload: output placeholder operands are dead (the NEFF
    output binds to the custom-call result buffer), one cached dummy is reused.
  - the jitted executable, device-resident weights, and placeholders are
    cached across calls, so a steady-state call is pure transfer + execute.

On-chip structure (per core, B_local=32):
  - x is fed pre-transposed as xT [C+1, S, BL] fp16 (row C = ones so b0 rides
    the matmul); per 32-step chunk one DMA stages it; per step a matmul pair
    computes the x-dependent backbone term straight into PSUM; the recurrent
    f32 matmul accumulates on top (no eviction/preload).
  - Persistent constants live in a single f32 "blob" (W0h, Wcat, bcat, ones)
    plus a small fp16 W0aug tensor and the f32 h0T state, one DMA each: the
    HW Matmult instruction tolerates a single semaphore wait, so three chained
    1x1 warm-up matmuls absorb the three DMA waits before any real matmul.
  - scan step: hT [128,32] -> MM1 accumulate -> ACT tanh [128,2,32] -> g;
    heads use g as the (P=32) stationary operand: psA=[ta|tb], psB=[ff1|d] in
    separate PSUM banks; per-bank K=1 ones-row matmuls add the biases
    (h-independent, off the critical path).
  - gate: DVE tensor_scalar (ta*-ts, PSUM->SBUF), DVE add (+tb), ACT sigmoid,
    DVE mul (*d), DVE add (+ff1) written into the f32 output staging tile; 4
    DVE 32x32 transposes produce hT for the next step.  At chunk end one ACT
    copy downcasts the staging tile to fp16 for the out DMA.
"""

import sys
import threading
from concurrent.futures import ThreadPoolExecutor

import numpy as np

for _p in ("/opt/trn_rl_repo",):
    if _p not in sys.path:
        sys.path.insert(0, _p)

B, S, C, U, H = 256, 2048, 64, 128, 256
NCORES = 8
BL = B // NCORES  # 32
CHUNK = 32
TS_SUPER = 256  # steps per timespan staging DMA
SEG = 256  # steps per device program (pipeline granularity)

# blob column layout (128 partitions x BLOB_COLS fp32)
_C_W0H = 0            # [128, 256]
_C_WCAT = 256         # [128, 1024] = 2 K-tiles x 512
_C_BC = 1280          # [1, 512] bcat (rows 1..127 stay zero -> zrow)
_C_ONES = 1792        # [1, 32] ones
BLOB_COLS = 1824


def _build_nc(s_total: int):
    import concourse.bass as bass
    import concourse.tile as tile
    from concourse import mybir
    from concourse.tile_rust import add_dep_helper
    import concourse.tile_sem_assignment as _tsa

    # All DMAs go through gpsimd/SWDGE; cap the SWDGE sem count so the
    # kernel-tail Drain's per-queue waits fit its struct's wait slots.
    _tsa.NUM_SWDGE_GLOBAL_SEMS = 2

    f32 = mybir.dt.float32
    f16 = mybir.dt.float16
    AF = mybir.ActivationFunctionType
    nchunk = s_total // CHUNK
    ts_super = min(TS_SUPER, s_total)

    nc = bass.Bass("TRN2")
    i8 = mybir.dt.int8
    xT_d = nc.dram_tensor("xT", [C + 1, s_total, BL], i8, kind="ExternalInput")
    nts_d = nc.dram_tensor("nts", [BL, s_total], f32, kind="ExternalInput")
    blob_d = nc.dram_tensor("blob", [128, BLOB_COLS], f32, kind="ExternalInput")
    w0a_d = nc.dram_tensor("w0a", [C + 1, H], f16, kind="ExternalInput")
    h0T_d = nc.dram_tensor("h0T", [U, BL], f32, kind="ExternalInput")
    out_d = nc.dram_tensor("out", [BL, s_total, U], f16, kind="ExternalOutput")
    hTo_d = nc.dram_tensor("hTout", [U, BL], f32, kind="ExternalOutput")

    with tile.TileContext(nc) as tc:
        with (
            tc.tile_pool(name="singles", bufs=1) as singles,
            tc.tile_pool(name="xstage", bufs=2) as xstage,
            tc.tile_pool(name="xfst", bufs=2) as xfstage,
            tc.tile_pool(name="tsstage", bufs=2) as tsstage,
            tc.tile_pool(name="outstage16", bufs=2) as outstage16,
            tc.tile_pool(name="ft", bufs=6) as ftp,
            tc.tile_pool(name="fb", bufs=6) as fbp,
            tc.tile_pool(name="gate", bufs=6) as gatep,
            tc.tile_pool(name="nh", bufs=3) as nhp,
            tc.tile_pool(name="ht", bufs=2) as htp,
            tc.tile_pool(name="psf", bufs=3, space="PSUM") as psfp,
            tc.tile_pool(name="psbnd", bufs=1, space="PSUM") as psbndp,
            tc.tile_pool(name="psa", bufs=2, space="PSUM") as psap,
            tc.tile_pool(name="psb", bufs=2, space="PSUM") as psbp,
        ):
            sb_blob = singles.tile([128, BLOB_COLS], f32, tag="blob")
            nc.gpsimd.dma_start(out=sb_blob, in_=blob_d[:, :])
            sb_w0a = singles.tile([C + 1, H], f16, tag="w0a")
            nc.gpsimd.dma_start(out=sb_w0a, in_=w0a_d[:, :])
            sb_h0T = singles.tile([U, BL], f32, tag="h0T")
            nc.gpsimd.dma_start(out=sb_h0T, in_=h0T_d[:, :])

            sb_W0h = sb_blob[:, _C_W0H : _C_W0H + H]
            sb_scr = singles.tile([1, 16], f32, tag="scratch")
            # a zero row of the blob: row 64 of the bcat column range (only
            # row 0 holds data there); base partition must be 0/32/64
            sb_zrow = sb_blob[64:65, _C_BC : _C_BC + 256]
            sb_bcat = sb_blob[0:1, _C_BC : _C_BC + 4 * U]
            sb_ones = sb_blob[0:1, _C_ONES : _C_ONES + BL]

            def wcat(k2, lo, hi):
                base = _C_WCAT + k2 * 4 * U
                return sb_blob[:, base + lo : base + hi]

            # warm-up: three 1x1 matmuls so PE observes each input DMA's
            # semaphore before any real matmul (Matmult carries at most one
            # sync wait); PE is in-order so they need no inter-deps.
            ps_w = psap.tile([BL, 2 * U], f32, tag="psa")
            nc.tensor.matmul(
                ps_w[0:1, 0:1], sb_blob[0:1, 0:1], sb_blob[0:1, 0:1],
                start=True, stop=True,
            )
            nc.tensor.matmul(
                ps_w[0:1, 0:1], sb_w0a[0:1, 0:1], sb_w0a[0:1, 0:1],
                start=True, stop=True,
            )
            nc.tensor.matmul(
                ps_w[0:1, 0:1], sb_h0T[0:1, 0:1], sb_h0T[0:1, 0:1],
                start=True, stop=True,
            )

            cur_hT = sb_h0T
            prev_pe = None  # last PE instruction of the previous step
            prev_act = None  # nosync chain pinning the ACT instruction order

            for ci in range(nchunk):
                s0 = ci * CHUNK
                xTa8 = xstage.tile([C + 1, CHUNK * BL], i8, tag="xta8")
                nc.gpsimd.dma_start(out=xTa8, in_=xT_d[:, s0 : s0 + CHUNK, :])
                xTa = xfstage.tile([C + 1, CHUNK * BL], f16, tag="xta")
                # ACT toucher: absorb the WAR vs this buffer's PE readers two
                # chunks back, so the convert below carries only the DMA wait
                xt_t = nc.scalar.copy(xTa[0:1, 0:1], sb_blob[0:1, 0:1])
                if prev_act is not None:
                    add_dep_helper(xt_t.ins, prev_act.ins, False, "act chain")
                prev_act = xt_t
                # dequantize int8 -> fp16 (scale is folded into w0a host-side)
                xt_c = nc.scalar.copy(xTa, xTa8)
                add_dep_helper(xt_c.ins, prev_act.ins, False, "act chain")
                prev_act = xt_c
                if s0 % ts_super == 0:
                    ntss = tsstage.tile([BL, ts_super], f32, tag="ntss")
                    nc.gpsimd.dma_start(out=ntss, in_=nts_d[:, s0 : s0 + ts_super])
                    # DVE toucher: absorb the ntss DMA wait on DVE once, so
                    # per-step tensor_scalar ops don't carry a second wait
                    sci = s0 // ts_super
                    nc.vector.tensor_copy(
                        sb_scr[0:1, sci : sci + 1], ntss[0:1, 0:1]
                    )

                ostage16 = outstage16.tile([BL, CHUNK * U], f16, tag="ostage16")
                # DVE toucher: absorb the WAR on the previous out-DMA of this
                # staging buffer so the per-step fp16 copies have only one wait
                nc.vector.memset(ostage16[0:1, 0:1], 0.0)

                for s in range(CHUNK):
                    st = (s0 + s) % ts_super  # index into ntss
                    # backbone: z = x-term + W0h.T @ hT, one accumulation group
                    # per m-tile (the x-term matmul is h-independent and runs
                    # ahead; same-group accumulation avoids extra PE waits)
                    # chunk-boundary step uses a dedicated psum tile: its
                    # slot-reuse WAW wait is then chunk-distant (dominated),
                    # leaving room for the xTa DMA wait (1-wait limit)
                    if s == 0:
                        ps_f = psbndp.tile([128, 2, BL], f32, tag="psbnd")
                    else:
                        ps_f = psfp.tile([128, 2, BL], f32, tag="psf")
                    # start=True clears the ENTIRE psum bank, so the two
                    # m-tiles (sharing one bank) must not each lead their own
                    # group: one K=1 zero-matmul clears/claims the whole
                    # region, everything else accumulates.
                    clr = nc.tensor.matmul(
                        ps_f,
                        sb_zrow[:, 0:128],
                        sb_zrow[:, 0 : 2 * BL],
                        start=True,
                        stop=False,
                        skip_group_check=True,
                    )
                    if prev_pe is not None:
                        add_dep_helper(clr.ins, prev_pe.ins, False, "clr after heads")
                    for m in range(2):
                        nc.tensor.matmul(
                            ps_f[:, m, :],
                            sb_w0a[:, m * 128 : (m + 1) * 128],
                            xTa[:, s * BL : (s + 1) * BL],
                            start=False,
                            stop=False,
                            skip_group_check=True,
                        )
                    mm1_last = None
                    for m in range(2):
                        mm1_last = nc.tensor.matmul(
                            ps_f[:, m, :],
                            sb_W0h[:, m * 128 : (m + 1) * 128],
                            cur_hT,
                            start=False,
                            stop=True,
                            skip_group_check=True,
                        )
                    # g = tanh(0.666 * z), both H-tiles in one ACT op
                    fT = ftp.tile([128, 2, BL], f32, tag="ft")
                    th = nc.scalar.activation(fT, ps_f, AF.Tanh, scale=0.666)
                    if prev_act is not None:
                        # nosync chain: fixes the ACT stream order so slot
                        # reuse stays outside the queue window and no ACT
                        # self-waits are emitted (Activation has 1 wait slot)
                        add_dep_helper(th.ins, prev_act.ins, False, "act chain")
                    prev_act = th

                    # heads: psA = [ta | tb], psB = [ff1 | d] (separate banks)
                    psA = psap.tile([BL, 2 * U], f32, tag="psa")
                    psB = psbp.tile([BL, 2 * U], f32, tag="psb")
                    # order-only dep: keep the bias matmuls behind this
                    # step's MM1 so their psum-WAR wait is dominated by MM1's
                    # DVE wait (Matmult tolerates only one sync wait)
                    bmA = nc.tensor.matmul(
                        psA, sb_ones, sb_bcat[:, 2 * U : 4 * U], start=True, stop=False
                    )
                    bmB = nc.tensor.matmul(
                        psB, sb_ones, sb_bcat[:, 0 : 2 * U], start=True, stop=False
                    )
                    add_dep_helper(bmA.ins, mm1_last.ins, False, "bias after MM1")
                    add_dep_helper(bmB.ins, mm1_last.ins, False, "bias after MM1")
                    for k2 in range(2):
                        nc.tensor.matmul(
                            psA,
                            fT[:, k2, :],
                            wcat(k2, 2 * U, 4 * U),
                            start=False,
                            stop=(k2 == 1),
                        )
                    for k2 in range(2):
                        prev_pe = nc.tensor.matmul(
                            psB,
                            fT[:, k2, :],
                            wcat(k2, 0, 2 * U),
                            start=False,
                            stop=(k2 == 1),
                        )

                    # gate: v = tb - ta*ts ; t = sigmoid(v) ; nh = ff1 + t*d
                    # (only one PSUM input allowed per DVE op). psB is evicted
                    # to SBUF on ACT (hidden behind t1/v) so t3's single ACT
                    # wait covers both the sigmoid and [ff1|d].
                    t1 = gatep.tile([BL, U], f32, tag="t1")
                    nc.vector.tensor_scalar_mul(t1, psA[:, 0:U], ntss[:, st : st + 1])
                    v = gatep.tile([BL, U], f32, tag="v")
                    nc.vector.tensor_add(v, t1, psA[:, U : 2 * U])
                    fB = fbp.tile([BL, 2 * U], f32, tag="fb")
                    cb = nc.scalar.copy(fB, psB)
                    add_dep_helper(cb.ins, prev_act.ins, False, "act chain")
                    prev_act = cb
                    sg = gatep.tile([BL, U], f32, tag="sg")
                    sgi = nc.scalar.activation(sg, v, AF.Sigmoid)
                    add_dep_helper(sgi.ins, prev_act.ins, False, "act chain")
                    prev_act = sgi
                    t3 = gatep.tile([BL, U], f32, tag="t3")
                    nc.vector.tensor_mul(t3, sg, fB[:, U : 2 * U])
                    nh = nhp.tile([BL, U], f32, tag="nh")
                    nc.vector.tensor_add(nh, t3, fB[:, 0:U])
                    # fp16 downcast into the output staging tile (DVE copy)
                    nc.vector.tensor_copy(
                        ostage16[:, s * U : (s + 1) * U], nh
                    )

                    # hT for the next step: 4x 32x32 DVE transposes
                    hT = htp.tile([U, BL], f32, tag="ht")
                    for j in range(4):
                        nc.vector.transpose(
                            hT[32 * j : 32 * (j + 1), :],
                            nh[:, 32 * j : 32 * (j + 1)],
                        )
                    cur_hT = hT

                nc.gpsimd.dma_start(out=out_d[:, s0 : s0 + CHUNK, :], in_=ostage16)

            # final hidden state (transposed) for segment chaining
            nc.gpsimd.dma_start(out=hTo_d[:, :], in_=cur_hT)

    _drop_stale_self_waits(nc, mybir)
    return nc


def _drop_stale_self_waits(nc, mybir, margin=8):
    """Compute instructions have a single usable wait slot (the engine-sem
    update takes the other).  Tile emits same-engine/same-lane waits for
    slot reuse even when the producer is far back; on an in-order engine or
    FIFO DMA lane those are redundant.  Drop self waits on instructions
    carrying >1 wait: engine-sem waits when >= `margin` instructions stale,
    own-DMA-lane waits always (the lane is FIFO)."""
    eng_prefix = {
        mybir.EngineType.PE: "PE",
        mybir.EngineType.DVE: "DVE",
        mybir.EngineType.Activation: "Activation",
        mybir.EngineType.Pool: "Pool",
        mybir.EngineType.SP: "SP",
    }
    tick = {}
    eng_ic = {}  # engine -> instruction count so far
    reach = {}  # sem name -> list of (value, engine_instr_idx) in order
    for fn in nc.m.functions:
        for blk in fn.blocks:
            for i in blk.instructions:
                si = i.sync_info
                if si is None:
                    continue
                eng = getattr(i, "engine", None)
                pfx = eng_prefix.get(eng)
                my_ic = eng_ic.get(eng, 0)
                upd_sems = {u.ant_name for u in si.on_update}
                if len(si.on_wait) > 1:
                    is_dma = type(i).__name__ == "InstDMACopy"
                    kept = []
                    for w in si.on_wait:
                        n = w.ant_name
                        if pfx and n.startswith(pfx + "_"):
                            # same-engine self-wait: redundant whenever the
                            # producing instruction precedes this one on the
                            # same in-order engine (Tile itself relies on
                            # program order for all same-engine hazards)
                            hist = reach.get(n, [])
                            prod_ic = None
                            for v, ic in reversed(hist):
                                if v >= w.wait_value:
                                    prod_ic = ic
                                else:
                                    break
                            if prod_ic is not None and prod_ic <= my_ic:
                                continue  # program-order-satisfied self-wait
                        if (
                            is_dma
                            and n in upd_sems
                            and ("DMASW" in n or "DMAHW" in n)
                            and tick.get(n, 0) >= w.wait_value
                        ):
                            continue  # own-lane FIFO wait
                        kept.append(w)
                    if len(kept) != len(si.on_wait):
                        si.on_wait = kept
                for u in si.on_update:
                    v = tick.get(u.ant_name, 0) + u.update_value
                    tick[u.ant_name] = v
                    reach.setdefault(u.ant_name, []).append((v, my_ic))
                eng_ic[eng] = my_ic + 1
    _split_multiwait_drains(nc, mybir)


def _split_multiwait_drains(nc, mybir):
    """The kernel-tail Drain waits on every engine/DMA-lane sem, but its
    struct has a single wait slot.  Split: inject one single-wait Drain per
    extra wait immediately before it on the same engine."""
    for fn in nc.m.functions:
        for blk in fn.blocks:
            insts = blk.instructions
            out = []
            changed = False
            for i in insts:
                si = i.sync_info
                if type(i).__name__ == "InstDrain" and si and len(si.on_wait) > 1:
                    waits = list(si.on_wait)
                    for k, w in enumerate(waits[:-1]):
                        d = mybir.InstDrain(name=f"{i.name}-w{k}", ins=[], outs=[])
                        d.engine = i.engine
                        d.sync_info = mybir.SyncInfo(on_wait=[w], on_update=[])
                        out.append(d)
                    si.on_wait = [waits[-1]]
                    changed = True
                out.append(i)
            if changed:
                blk.instructions = out


def _prep_weights(W0, b0, W1, b1, W2, b2, Wa, ba, Wb, bb):
    W0 = np.asarray(W0, np.float32)
    W0x = W0[:C] / 100.0
    W0h = np.ascontiguousarray(W0[C:])  # [U, H]
    b0p = np.asarray(b0, np.float32) - 0.65 * W0[:C].sum(axis=0)
    W0aug = np.concatenate([W0x, b0p[None, :]], axis=0)  # [C+1, H]
    a = np.float32(1.7159)
    Wcat = np.concatenate([a * W1, a * (W2 - W1), a * Wa, a * Wb], axis=1)  # [H, 4U]
    bcat = np.concatenate([b1, b2 - b1, ba, bb]).astype(np.float32)  # [4U]
    return (
        W0aug.astype(np.float32),
        W0h.astype(np.float32),
        Wcat.astype(np.float32),
        bcat,
    )


def _make_blob(weights):
    W0aug, W0h, Wcat, bcat = weights
    blob = np.zeros((128, BLOB_COLS), np.float32)
    blob[:, _C_W0H : _C_W0H + H] = W0h
    for k2 in range(2):
        blob[:, _C_WCAT + k2 * 4 * U : _C_WCAT + (k2 + 1) * 4 * U] = Wcat[
            k2 * 128 : (k2 + 1) * 128, :
        ]
    blob[0, _C_BC : _C_BC + 4 * U] = bcat
    blob[0, _C_ONES : _C_ONES + BL] = 1.0
    return blob


class _Res:
    exec_time_ns = None
    mean_exec_time_ns = None
    instructions_and_trace = None
    profile_json = None


_CACHE = {}
_CACHE_LOCK = threading.Lock()


def _get_rt(s_seg):
    """Build (once) the Bass program + jitted sharded callable for a segment
    length, plus cached device-resident output placeholders."""
    key = ("rt", s_seg)
    with _CACHE_LOCK:
        if key in _CACHE:
            return _CACHE[key]
    import jax
    from jax.sharding import Mesh, PartitionSpec, NamedSharding
    from jax.experimental.shard_map import shard_map
    from concourse import mybir
    from concourse.bass2jax import (
        _bass_exec_p,
        install_neuronx_cc_hook,
        partition_id_tensor,
    )

    install_neuronx_cc_hook()
    nc = _build_nc(s_seg)

    in_names, out_names, out_avals = [], [], []
    for alloc in nc.m.functions[0].allocations:
        if not isinstance(alloc, mybir.MemoryLocationSet):
            continue
        name = alloc.memorylocations[0].name
        if alloc.kind == "ExternalInput":
            in_names.append(name)
        elif alloc.kind == "ExternalOutput":
            out_names.append(name)
            out_avals.append(
                jax.core.ShapedArray(
                    tuple(alloc.tensor_shape), mybir.dt.np(alloc.dtype)
                )
            )
    partition_name = nc.partition_id_tensor.name if nc.partition_id_tensor else None
    if partition_name is not None:
        in_names.remove(partition_name)
    all_in = in_names + out_names

    def _body(*args):
        operands = list(args)
        if partition_name is not None:
            operands.append(partition_id_tensor())
        outs = _bass_exec_p.bind(
            *operands,
            out_avals=tuple(out_avals),
            in_names=tuple(all_in + ([partition_name] if partition_name else [])),
            out_names=tuple(out_names),
            lowering_input_output_aliases=(),
            sim_require_finite=True,
            sim_require_nnan=True,
            nc=nc,
        )
        return tuple(outs)

    devices = jax.devices()[:NCORES]
    mesh = Mesh(np.asarray(devices), ("core",))
    P = PartitionSpec
    jitted = jax.jit(
        shard_map(
            _body,
            mesh=mesh,
            in_specs=(P("core"),) * len(all_in),
            out_specs=(P("core"),) * len(out_names),
            check_rep=False,
        ),
        keep_unused=True,
    )
    sh = NamedSharding(mesh, P("core"))
    ph_out = jax.device_put(np.zeros((NCORES * BL, s_seg, U), np.float16), sh)
    ph_hT = jax.device_put(np.zeros((NCORES * U, BL), np.float32), sh)
    rt = dict(
        nc=nc, jitted=jitted, sh=sh, in_names=in_names, out_names=out_names,
        ph_out=ph_out, ph_hT=ph_hT,
    )
    with _CACHE_LOCK:
        _CACHE[key] = rt
    return rt


def _weights_dev(weights, sh):
    """Device-resident replicated blob, cached per weights object."""
    import jax

    key = ("wdev", id(weights))
    with _CACHE_LOCK:
        hit = _CACHE.get(key)
    if hit is not None:
        return hit
    blob = _make_blob(weights)  # [128, BLOB_COLS] f32
    blob_g = np.broadcast_to(blob, (NCORES, *blob.shape)).reshape(
        NCORES * 128, BLOB_COLS
    )
    blob_dev = jax.device_put(np.ascontiguousarray(blob_g), sh)
    with _CACHE_LOCK:
        _CACHE[key] = blob_dev
    return blob_dev


def _w0a_dev(weights, xmax, sh):
    """Per-call fp16 W0aug with the int8 dequant scale folded into the
    x rows (33KB upload)."""
    import jax

    w0a = weights[0].copy()  # [C+1, H] f32
    w0a[:C] *= np.float32(xmax / 127.0)
    w0a_g = np.broadcast_to(w0a.astype(np.float16), (NCORES, C + 1, H)).reshape(
        NCORES * (C + 1), H
    )
    return jax.device_put(np.ascontiguousarray(w0a_g), sh)


def _prep_x_seg(xf, sc, s0, s_seg):
    """Quantize + transpose one segment: [B, S, C] f32 -> [8*(C+1), s_seg, BL]
    int8 (done per segment so it hides under the wire transfers)."""
    xg = np.empty((NCORES * (C + 1), s_seg, BL), np.int8)
    q = np.clip(np.rint(xf[:, s0 : s0 + s_seg, :] * sc), -127, 127).astype(np.int8)
    for c in range(NCORES):
        blk = xg[c * (C + 1) : (c + 1) * (C + 1)]
        blk[:C] = q[c * BL : (c + 1) * BL].transpose(2, 1, 0)
        blk[C] = 1  # ones plane carries the (unscaled) bias row
    return xg


def _fetch_shard(full, s0, s_seg, shard):
    c = shard.index[0].start // BL
    arr = np.asarray(shard.data)  # [BL, s_seg, U] f16 (blocks until ready)
    full[c * BL : (c + 1) * BL, s0 : s0 + s_seg] = arr  # f16 -> f32 cast


def run(x_codes, h0, timespans, weights, s_total=S, trace=False):
    import jax

    s_seg = min(SEG, s_total)
    assert s_total % s_seg == 0 and s_seg % CHUNK == 0
    nseg = s_total // s_seg
    rt = _get_rt(s_seg)
    blob_dev = _weights_dev(weights, rt["sh"])

    xf = np.asarray(x_codes, np.float32)[:, :s_total]
    xmax = float(max(xf.max(), -float(xf.min()), 1e-30))
    w0a_dev = _w0a_dev(weights, xmax, rt["sh"])
    sc = np.float32(127.0 / xmax)
    nts = -np.asarray(timespans, np.float32)[:, :s_total]
    h0 = np.asarray(h0, np.float32)
    h0T_g = np.ascontiguousarray(
        h0.reshape(NCORES, BL, U).transpose(0, 2, 1).reshape(NCORES * U, BL)
    )
    h_cur = jax.device_put(h0T_g, rt["sh"])

    by_name_static = {"blob": blob_dev, "w0a": w0a_dev}
    full = np.empty((B, s_total, U), np.float32)
    futs = []
    with ThreadPoolExecutor(max_workers=8) as ex:
        for si in range(nseg):
            s0 = si * s_seg
            xg = _prep_x_seg(xf, sc, s0, s_seg)
            ng = np.ascontiguousarray(nts[:, s0 : s0 + s_seg])
            x_dev = jax.device_put(xg, rt["sh"])
            n_dev = jax.device_put(ng, rt["sh"])
            by_name = {
                **by_name_static,
                "xT": x_dev, "nts": n_dev, "h0T": h_cur,
                "out": rt["ph_out"], "hTout": rt["ph_hT"],
            }
            args = [by_name[n] for n in rt["in_names"] + rt["out_names"]]
            res = rt["jitted"](*args)
            outs = dict(zip(rt["out_names"], res))
            h_cur = outs["hTout"]
            for shard in outs["out"].addressable_shards:
                futs.append(ex.submit(_fetch_shard, full, s0, s_seg, shard))
        for f in futs:
            f.result()
    return full, _Res()


def kernel(x_codes, h0, timespans, W0, b0, W1, b1, W2, b2, Wa, ba, Wb, bb):
    weights = _prep_weights(W0, b0, W1, b1, W2, b2, Wa, ba, Wb, bb)
    full, _ = run(
        np.asarray(x_codes, np.float32),
        np.asarray(h0, np.float32),
        np.asarray(timespans, np.float32),
        weights,
        S,
    )
    return full.astype(np.float32)


# revision 13
# speedup vs baseline: 1.2139x; 1.0108x over previous
"""CfC RNN scan kernel for Trainium2 (8 NeuronCores, data-parallel over batch).

Math (per step, from the reference):
    f   = 1.7159 * tanh(0.666 * (concat(x_s, h) @ W0 + b0))     x_s = (x-65)/100
    ff1 = f @ W1 + b1 ;  ff2 = f @ W2 + b2
    ta  = f @ Wa + ba ;  tb  = f @ Wb + bb
    t   = sigmoid(tb - ta * ts)
    h'  = ff1 + t * (ff2 - ff1)

Folding done on the host:
  - input scale/shift folded into W0x, b0:  xterm = x @ (W0x/100) + (b0 - .65*W0x.sum(0))
  - 1.7159 folded into the head weights; heads consume g = tanh(0.666*z) directly
  - d = ff2-ff1 computed via Wd = W2-W1, bd = b2-b1
  - head weights concatenated: Wcat = [W1' | Wd' | Wa' | Wb'] (256 x 512)

End-to-end: the wall-clock here is dominated by the axon tunnel (~60MB/s up,
~35MB/s down), so the runner minimizes wire bytes and overlaps transfers:
  - x ships as fp16 [C+1, S, BL] (half the bytes); the x-term matmuls run in
    fp16 against an fp16 copy of W0aug (error ~1e-5 on z, way under tol).
  - the output ships as fp16 (one ACT downcast per 32-step chunk on-device),
    upcast to f32 on the host. Per-element rel err ~5e-4 vs 2e-2 tolerance.
  - the scan is cut into SEG-step segments chained through a device-resident
    hT state tensor; uploads of segment i+1 and downloads of segment i overlap
    via jax async dispatch + background fetch threads.
  - no 256MB zero-init upload: output placeholder operands are dead (the NEFF
    output binds to the custom-call result buffer), one cached dummy is reused.
  - the jitted executable, device-resident weights, and placeholders are
    cached across calls, so a steady-state call is pure transfer + execute.

On-chip structure (per core, B_local=32):
  - x is fed pre-transposed as xT [C+1, S, BL] fp16 (row C = ones so b0 rides
    the matmul); per 32-step chunk one DMA stages it; per step a matmul pair
    computes the x-dependent backbone term straight into PSUM; the recurrent
    f32 matmul accumulates on top (no eviction/preload).
  - Persistent constants live in a single f32 "blob" (W0h, Wcat, bcat, ones)
    plus a small fp16 W0aug tensor and the f32 h0T state, one DMA each: the
    HW Matmult instruction tolerates a single semaphore wait, so three chained
    1x1 warm-up matmuls absorb the three DMA waits before any real matmul.
  - scan step: hT [128,32] -> MM1 accumulate -> ACT tanh [128,2,32] -> g;
    heads use g as the (P=32) stationary operand: psA=[ta|tb], psB=[ff1|d] in
    separate PSUM banks; per-bank K=1 ones-row matmuls add the biases
    (h-independent, off the critical path).
  - gate: DVE tensor_scalar (ta*-ts, PSUM->SBUF), DVE add (+tb), ACT sigmoid,
    DVE mul (*d), DVE add (+ff1) written into the f32 output staging tile; 4
    DVE 32x32 transposes produce hT for the next step.  At chunk end one ACT
    copy downcasts the staging tile to fp16 for the out DMA.
"""

import sys
import threading
from concurrent.futures import ThreadPoolExecutor

import numpy as np

for _p in ("/opt/trn_rl_repo",):
    if _p not in sys.path:
        sys.path.insert(0, _p)

B, S, C, U, H = 256, 2048, 64, 128, 256
NCORES = 8
BL = B // NCORES  # 32
CHUNK = 32
TS_SUPER = 256  # steps per timespan staging DMA
SEG = 256  # steps per device program (pipeline granularity)

# blob column layout (128 partitions x BLOB_COLS fp32)
_C_W0H = 0            # [128, 256]
_C_WCAT = 256         # [128, 1024] = 2 K-tiles x 512
_C_BC = 1280          # [1, 512] bcat (rows 1..127 stay zero -> zrow)
_C_ONES = 1792        # [1, 32] ones
BLOB_COLS = 1824


def _build_nc(s_total: int):
    import concourse.bass as bass
    import concourse.tile as tile
    from concourse import mybir
    from concourse.tile_rust import add_dep_helper
    import concourse.tile_sem_assignment as _tsa

    # All DMAs go through gpsimd/SWDGE; cap the SWDGE sem count so the
    # kernel-tail Drain's per-queue waits fit its struct's wait slots.
    _tsa.NUM_SWDGE_GLOBAL_SEMS = 2

    f32 = mybir.dt.float32
    f16 = mybir.dt.float16
    AF = mybir.ActivationFunctionType
    nchunk = s_total // CHUNK
    ts_super = min(TS_SUPER, s_total)

    nc = bass.Bass("TRN2")
    i8 = mybir.dt.int8
    xT_d = nc.dram_tensor("xT", [C + 1, s_total, BL], i8, kind="ExternalInput")
    nts_d = nc.dram_tensor("nts", [BL, s_total], f32, kind="ExternalInput")
    blob_d = nc.dram_tensor("blob", [128, BLOB_COLS], f32, kind="ExternalInput")
    w0a_d = nc.dram_tensor("w0a", [C + 1, H], f16, kind="ExternalInput")
    h0T_d = nc.dram_tensor("h0T", [U, BL], f32, kind="ExternalInput")
    out_d = nc.dram_tensor("out", [BL, s_total, U], f16, kind="ExternalOutput")
    hTo_d = nc.dram_tensor("hTout", [U, BL], f32, kind="ExternalOutput")

    with tile.TileContext(nc) as tc:
        with (
            tc.tile_pool(name="singles", bufs=1) as singles,
            tc.tile_pool(name="xstage", bufs=2) as xstage,
            tc.tile_pool(name="xfst", bufs=2) as xfstage,
            tc.tile_pool(name="tsstage", bufs=2) as tsstage,
            tc.tile_pool(name="outstage16", bufs=2) as outstage16,
            tc.tile_pool(name="ft", bufs=6) as ftp,
            tc.tile_pool(name="fb", bufs=6) as fbp,
            tc.tile_pool(name="gate", bufs=6) as gatep,
            tc.tile_pool(name="nh", bufs=3) as nhp,
            tc.tile_pool(name="ht", bufs=2) as htp,
            tc.tile_pool(name="psf", bufs=3, space="PSUM") as psfp,
            tc.tile_pool(name="psbnd", bufs=1, space="PSUM") as psbndp,
            tc.tile_pool(name="psa", bufs=2, space="PSUM") as psap,
            tc.tile_pool(name="psb", bufs=2, space="PSUM") as psbp,
        ):
            sb_blob = singles.tile([128, BLOB_COLS], f32, tag="blob")
            nc.gpsimd.dma_start(out=sb_blob, in_=blob_d[:, :])
            sb_w0a = singles.tile([C + 1, H], f16, tag="w0a")
            nc.gpsimd.dma_start(out=sb_w0a, in_=w0a_d[:, :])
            sb_h0T = singles.tile([U, BL], f32, tag="h0T")
            nc.gpsimd.dma_start(out=sb_h0T, in_=h0T_d[:, :])

            sb_W0h = sb_blob[:, _C_W0H : _C_W0H + H]
            sb_scr = singles.tile([1, 16], f32, tag="scratch")
            # a zero row of the blob: row 64 of the bcat column range (only
            # row 0 holds data there); base partition must be 0/32/64
            sb_zrow = sb_blob[64:65, _C_BC : _C_BC + 256]
            sb_bcat = sb_blob[0:1, _C_BC : _C_BC + 4 * U]
            sb_ones = sb_blob[0:1, _C_ONES : _C_ONES + BL]

            def wcat(k2, lo, hi):
                base = _C_WCAT + k2 * 4 * U
                return sb_blob[:, base + lo : base + hi]

            # warm-up: three 1x1 matmuls so PE observes each input DMA's
            # semaphore before any real matmul (Matmult carries at most one
            # sync wait); PE is in-order so they need no inter-deps.
            ps_w = psap.tile([BL, 2 * U], f32, tag="psa")
            nc.tensor.matmul(
                ps_w[0:1, 0:1], sb_blob[0:1, 0:1], sb_blob[0:1, 0:1],
                start=True, stop=True,
            )
            nc.tensor.matmul(
                ps_w[0:1, 0:1], sb_w0a[0:1, 0:1], sb_w0a[0:1, 0:1],
                start=True, stop=True,
            )
            nc.tensor.matmul(
                ps_w[0:1, 0:1], sb_h0T[0:1, 0:1], sb_h0T[0:1, 0:1],
                start=True, stop=True,
            )

            cur_hT = sb_h0T
            prev_pe = None  # last PE instruction of the previous step
            prev_act = None  # nosync chain pinning the ACT instruction order

            for ci in range(nchunk):
                s0 = ci * CHUNK
                xTa8 = xstage.tile([C + 1, CHUNK * BL], i8, tag="xta8")
                nc.gpsimd.dma_start(out=xTa8, in_=xT_d[:, s0 : s0 + CHUNK, :])
                xTa = xfstage.tile([C + 1, CHUNK * BL], f16, tag="xta")
                # ACT toucher: absorb the WAR vs this buffer's PE readers two
                # chunks back, so the convert below carries only the DMA wait
                xt_t = nc.scalar.copy(xTa[0:1, 0:1], sb_blob[0:1, 0:1])
                if prev_act is not None:
                    add_dep_helper(xt_t.ins, prev_act.ins, False, "act chain")
                prev_act = xt_t
                # dequantize int8 -> fp16 (scale is folded into w0a host-side)
                xt_c = nc.scalar.copy(xTa, xTa8)
                add_dep_helper(xt_c.ins, prev_act.ins, False, "act chain")
                prev_act = xt_c
                if s0 % ts_super == 0:
                    ntss = tsstage.tile([BL, ts_super], f32, tag="ntss")
                    nc.gpsimd.dma_start(out=ntss, in_=nts_d[:, s0 : s0 + ts_super])
                    # DVE toucher: absorb the ntss DMA wait on DVE once, so
                    # per-step tensor_scalar ops don't carry a second wait
                    sci = s0 // ts_super
                    nc.vector.tensor_copy(
                        sb_scr[0:1, sci : sci + 1], ntss[0:1, 0:1]
                    )

                ostage16 = outstage16.tile([BL, CHUNK * U], f16, tag="ostage16")
                # DVE toucher: absorb the WAR on the previous out-DMA of this
                # staging buffer so the per-step fp16 copies have only one wait
                nc.vector.memset(ostage16[0:1, 0:1], 0.0)

                for s in range(CHUNK):
                    st = (s0 + s) % ts_super  # index into ntss
                    # backbone: z = x-term + W0h.T @ hT, one accumulation group
                    # per m-tile (the x-term matmul is h-independent and runs
                    # ahead; same-group accumulation avoids extra PE waits)
                    # chunk-boundary step uses a dedicated psum tile: its
                    # slot-reuse WAW wait is then chunk-distant (dominated),
                    # leaving room for the xTa DMA wait (1-wait limit)
                    if s == 0:
                        ps_f = psbndp.tile([128, 2, BL], f32, tag="psbnd")
                    else:
                        ps_f = psfp.tile([128, 2, BL], f32, tag="psf")
                    # start=True clears the ENTIRE psum bank, so the two
                    # m-tiles (sharing one bank) must not each lead their own
                    # group: one K=1 zero-matmul clears/claims the whole
                    # region, everything else accumulates.
                    clr = nc.tensor.matmul(
                        ps_f,
                        sb_zrow[:, 0:128],
                        sb_zrow[:, 0 : 2 * BL],
                        start=True,
                        stop=False,
                        skip_group_check=True,
                    )
                    if prev_pe is not None:
                        add_dep_helper(clr.ins, prev_pe.ins, False, "clr after heads")
                    for m in range(2):
                        nc.tensor.matmul(
                            ps_f[:, m, :],
                            sb_w0a[:, m * 128 : (m + 1) * 128],
                            xTa[:, s * BL : (s + 1) * BL],
                            start=False,
                            stop=False,
                            skip_group_check=True,
                        )
                    mm1_last = None
                    for m in range(2):
                        mm1_last = nc.tensor.matmul(
                            ps_f[:, m, :],
                            sb_W0h[:, m * 128 : (m + 1) * 128],
                            cur_hT,
                            start=False,
                            stop=True,
                            skip_group_check=True,
                        )
                    # g = tanh(0.666 * z), both H-tiles in one ACT op
                    fT = ftp.tile([128, 2, BL], f32, tag="ft")
                    th = nc.scalar.activation(fT, ps_f, AF.Tanh, scale=0.666)
                    if prev_act is not None:
                        # nosync chain: fixes the ACT stream order so slot
                        # reuse stays outside the queue window and no ACT
                        # self-waits are emitted (Activation has 1 wait slot)
                        add_dep_helper(th.ins, prev_act.ins, False, "act chain")
                    prev_act = th

                    # heads: psA = [ta | tb], psB = [ff1 | d] (separate banks)
                    psA = psap.tile([BL, 2 * U], f32, tag="psa")
                    psB = psbp.tile([BL, 2 * U], f32, tag="psb")
                    # order-only dep: keep the bias matmuls behind this
                    # step's MM1 so their psum-WAR wait is dominated by MM1's
                    # DVE wait (Matmult tolerates only one sync wait)
                    bmA = nc.tensor.matmul(
                        psA, sb_ones, sb_bcat[:, 2 * U : 4 * U], start=True, stop=False
                    )
                    bmB = nc.tensor.matmul(
                        psB, sb_ones, sb_bcat[:, 0 : 2 * U], start=True, stop=False
                    )
                    add_dep_helper(bmA.ins, mm1_last.ins, False, "bias after MM1")
                    add_dep_helper(bmB.ins, mm1_last.ins, False, "bias after MM1")
                    for k2 in range(2):
                        nc.tensor.matmul(
                            psA,
                            fT[:, k2, :],
                            wcat(k2, 2 * U, 4 * U),
                            start=False,
                            stop=(k2 == 1),
                        )
                    for k2 in range(2):
                        prev_pe = nc.tensor.matmul(
                            psB,
                            fT[:, k2, :],
                            wcat(k2, 0, 2 * U),
                            start=False,
                            stop=(k2 == 1),
                        )

                    # gate: v = tb - ta*ts ; t = sigmoid(v) ; nh = ff1 + t*d
                    # (only one PSUM input allowed per DVE op). psB is evicted
                    # to SBUF on ACT (hidden behind t1/v) so t3's single ACT
                    # wait covers both the sigmoid and [ff1|d].
                    t1 = gatep.tile([BL, U], f32, tag="t1")
                    nc.vector.tensor_scalar_mul(t1, psA[:, 0:U], ntss[:, st : st + 1])
                    v = gatep.tile([BL, U], f32, tag="v")
                    nc.vector.tensor_add(v, t1, psA[:, U : 2 * U])
                    fB = fbp.tile([BL, 2 * U], f32, tag="fb")
                    cb = nc.scalar.copy(fB, psB)
                    add_dep_helper(cb.ins, prev_act.ins, False, "act chain")
                    prev_act = cb
                    sg = gatep.tile([BL, U], f32, tag="sg")
                    sgi = nc.scalar.activation(sg, v, AF.Sigmoid)
                    add_dep_helper(sgi.ins, prev_act.ins, False, "act chain")
                    prev_act = sgi
                    t3 = gatep.tile([BL, U], f32, tag="t3")
                    nc.vector.tensor_mul(t3, sg, fB[:, U : 2 * U])
                    nh = nhp.tile([BL, U], f32, tag="nh")
                    nc.vector.tensor_add(nh, t3, fB[:, 0:U])
                    # fp16 downcast into the output staging tile (DVE copy)
                    nc.vector.tensor_copy(
                        ostage16[:, s * U : (s + 1) * U], nh
                    )

                    # hT for the next step: 4x 32x32 DVE transposes
                    hT = htp.tile([U, BL], f32, tag="ht")
                    for j in range(4):
                        nc.vector.transpose(
                            hT[32 * j : 32 * (j + 1), :],
                            nh[:, 32 * j : 32 * (j + 1)],
                        )
                    cur_hT = hT

                nc.gpsimd.dma_start(out=out_d[:, s0 : s0 + CHUNK, :], in_=ostage16)

            # final hidden state (transposed) for segment chaining
            nc.gpsimd.dma_start(out=hTo_d[:, :], in_=cur_hT)

    _drop_stale_self_waits(nc, mybir)
    return nc


def _drop_stale_self_waits(nc, mybir, margin=8):
    """Compute instructions have a single usable wait slot (the engine-sem
    update takes the other).  Tile emits same-engine/same-lane waits for
    slot reuse even when the producer is far back; on an in-order engine or
    FIFO DMA lane those are redundant.  Drop self waits on instructions
    carrying >1 wait: engine-sem waits when >= `margin` instructions stale,
    own-DMA-lane waits always (the lane is FIFO)."""
    eng_prefix = {
        mybir.EngineType.PE: "PE",
        mybir.EngineType.DVE: "DVE",
        mybir.EngineType.Activation: "Activation",
        mybir.EngineType.Pool: "Pool",
        mybir.EngineType.SP: "SP",
    }
    tick = {}
    eng_ic = {}  # engine -> instruction count so far
    reach = {}  # sem name -> list of (value, engine_instr_idx) in order
    for fn in nc.m.functions:
        for blk in fn.blocks:
            for i in blk.instructions:
                si = i.sync_info
                if si is None:
                    continue
                eng = getattr(i, "engine", None)
                pfx = eng_prefix.get(eng)
                my_ic = eng_ic.get(eng, 0)
                upd_sems = {u.ant_name for u in si.on_update}
                if len(si.on_wait) > 1:
                    is_dma = type(i).__name__ == "InstDMACopy"
                    kept = []
                    for w in si.on_wait:
                        n = w.ant_name
                        if pfx and n.startswith(pfx + "_"):
                            # same-engine self-wait: redundant whenever the
                            # producing instruction precedes this one on the
                            # same in-order engine (Tile itself relies on
                            # program order for all same-engine hazards)
                            hist = reach.get(n, [])
                            prod_ic = None
                            for v, ic in reversed(hist):
                                if v >= w.wait_value:
                                    prod_ic = ic
                                else:
                                    break
                            if prod_ic is not None and prod_ic <= my_ic:
                                continue  # program-order-satisfied self-wait
                        if (
                            is_dma
                            and n in upd_sems
                            and ("DMASW" in n or "DMAHW" in n)
                            and tick.get(n, 0) >= w.wait_value
                        ):
                            continue  # own-lane FIFO wait
                        kept.append(w)
                    if len(kept) != len(si.on_wait):
                        si.on_wait = kept
                for u in si.on_update:
                    v = tick.get(u.ant_name, 0) + u.update_value
                    tick[u.ant_name] = v
                    reach.setdefault(u.ant_name, []).append((v, my_ic))
                eng_ic[eng] = my_ic + 1
    _split_multiwait_drains(nc, mybir)


def _split_multiwait_drains(nc, mybir):
    """The kernel-tail Drain waits on every engine/DMA-lane sem, but its
    struct has a single wait slot.  Split: inject one single-wait Drain per
    extra wait immediately before it on the same engine."""
    for fn in nc.m.functions:
        for blk in fn.blocks:
            insts = blk.instructions
            out = []
            changed = False
            for i in insts:
                si = i.sync_info
                if type(i).__name__ == "InstDrain" and si and len(si.on_wait) > 1:
                    waits = list(si.on_wait)
                    for k, w in enumerate(waits[:-1]):
                        d = mybir.InstDrain(name=f"{i.name}-w{k}", ins=[], outs=[])
                        d.engine = i.engine
                        d.sync_info = mybir.SyncInfo(on_wait=[w], on_update=[])
                        out.append(d)
                    si.on_wait = [waits[-1]]
                    changed = True
                out.append(i)
            if changed:
                blk.instructions = out


def _prep_weights(W0, b0, W1, b1, W2, b2, Wa, ba, Wb, bb):
    W0 = np.asarray(W0, np.float32)
    W0x = W0[:C] / 100.0
    W0h = np.ascontiguousarray(W0[C:])  # [U, H]
    b0p = np.asarray(b0, np.float32) - 0.65 * W0[:C].sum(axis=0)
    W0aug = np.concatenate([W0x, b0p[None, :]], axis=0)  # [C+1, H]
    a = np.float32(1.7159)
    Wcat = np.concatenate([a * W1, a * (W2 - W1), a * Wa, a * Wb], axis=1)  # [H, 4U]
    bcat = np.concatenate([b1, b2 - b1, ba, bb]).astype(np.float32)  # [4U]
    return (
        W0aug.astype(np.float32),
        W0h.astype(np.float32),
        Wcat.astype(np.float32),
        bcat,
    )


def _make_blob(weights):
    W0aug, W0h, Wcat, bcat = weights
    blob = np.zeros((128, BLOB_COLS), np.float32)
    blob[:, _C_W0H : _C_W0H + H] = W0h
    for k2 in range(2):
        blob[:, _C_WCAT + k2 * 4 * U : _C_WCAT + (k2 + 1) * 4 * U] = Wcat[
            k2 * 128 : (k2 + 1) * 128, :
        ]
    blob[0, _C_BC : _C_BC + 4 * U] = bcat
    blob[0, _C_ONES : _C_ONES + BL] = 1.0
    return blob


class _Res:
    exec_time_ns = None
    mean_exec_time_ns = None
    instructions_and_trace = None
    profile_json = None


_CACHE = {}
_CACHE_LOCK = threading.Lock()


def _get_rt(s_seg):
    """Build (once) the Bass program + jitted sharded callable for a segment
    length, plus cached device-resident output placeholders."""
    key = ("rt", s_seg)
    with _CACHE_LOCK:
        if key in _CACHE:
            return _CACHE[key]
    import jax
    from jax.sharding import Mesh, PartitionSpec, NamedSharding
    from jax.experimental.shard_map import shard_map
    from concourse import mybir
    from concourse.bass2jax import (
        _bass_exec_p,
        install_neuronx_cc_hook,
        partition_id_tensor,
    )

    install_neuronx_cc_hook()
    nc = _build_nc(s_seg)

    in_names, out_names, out_avals = [], [], []
    for alloc in nc.m.functions[0].allocations:
        if not isinstance(alloc, mybir.MemoryLocationSet):
            continue
        name = alloc.memorylocations[0].name
        if alloc.kind == "ExternalInput":
            in_names.append(name)
        elif alloc.kind == "ExternalOutput":
            out_names.append(name)
            out_avals.append(
                jax.core.ShapedArray(
                    tuple(alloc.tensor_shape), mybir.dt.np(alloc.dtype)
                )
            )
    partition_name = nc.partition_id_tensor.name if nc.partition_id_tensor else None
    if partition_name is not None:
        in_names.remove(partition_name)
    all_in = in_names + out_names

    def _body(*args):
        operands = list(args)
        if partition_name is not None:
            operands.append(partition_id_tensor())
        outs = _bass_exec_p.bind(
            *operands,
            out_avals=tuple(out_avals),
            in_names=tuple(all_in + ([partition_name] if partition_name else [])),
            out_names=tuple(out_names),
            lowering_input_output_aliases=(),
            sim_require_finite=True,
            sim_require_nnan=True,
            nc=nc,
        )
        return tuple(outs)

    devices = jax.devices()[:NCORES]
    mesh = Mesh(np.asarray(devices), ("core",))
    P = PartitionSpec
    jitted = jax.jit(
        shard_map(
            _body,
            mesh=mesh,
            in_specs=(P("core"),) * len(all_in),
            out_specs=(P("core"),) * len(out_names),
            check_rep=False,
        ),
        keep_unused=True,
    )
    sh = NamedSharding(mesh, P("core"))
    ph_out = jax.device_put(np.zeros((NCORES * BL, s_seg, U), np.float16), sh)
    ph_hT = jax.device_put(np.zeros((NCORES * U, BL), np.float32), sh)
    rt = dict(
        nc=nc, jitted=jitted, sh=sh, in_names=in_names, out_names=out_names,
        ph_out=ph_out, ph_hT=ph_hT,
    )
    with _CACHE_LOCK:
        _CACHE[key] = rt
    return rt


def _weights_dev(weights, sh):
    """Device-resident replicated blob, cached per weights object."""
    import jax

    key = ("wdev", id(weights))
    with _CACHE_LOCK:
        hit = _CACHE.get(key)
    if hit is not None and hit[0] is weights:
        return hit[1]
    blob = _make_blob(weights)  # [128, BLOB_COLS] f32
    blob_g = np.broadcast_to(blob, (NCORES, *blob.shape)).reshape(
        NCORES * 128, BLOB_COLS
    )
    blob_dev = jax.device_put(np.ascontiguousarray(blob_g), sh)
    with _CACHE_LOCK:
        # hold a ref to `weights` so its id can't be recycled into a stale hit
        _CACHE[key] = (weights, blob_dev)
    return blob_dev


def _w0a_dev(weights, xmax, sh):
    """Per-call fp16 W0aug with the int8 dequant scale folded into the
    x rows (33KB upload)."""
    import jax

    w0a = weights[0].copy()  # [C+1, H] f32
    w0a[:C] *= np.float32(xmax / 127.0)
    w0a_g = np.broadcast_to(w0a.astype(np.float16), (NCORES, C + 1, H)).reshape(
        NCORES * (C + 1), H
    )
    return jax.device_put(np.ascontiguousarray(w0a_g), sh)


def _prep_x_seg(xf, sc, s0, s_seg):
    """Quantize + transpose one segment: [B, S, C] f32 -> [8*(C+1), s_seg, BL]
    int8 (done per segment so it hides under the wire transfers)."""
    xg = np.empty((NCORES * (C + 1), s_seg, BL), np.int8)
    q = np.clip(np.rint(xf[:, s0 : s0 + s_seg, :] * sc), -127, 127).astype(np.int8)
    for c in range(NCORES):
        blk = xg[c * (C + 1) : (c + 1) * (C + 1)]
        blk[:C] = q[c * BL : (c + 1) * BL].transpose(2, 1, 0)
        blk[C] = 1  # ones plane carries the (unscaled) bias row
    return xg


def _fetch_shard(full, s0, s_seg, shard):
    c = shard.index[0].start // BL
    arr = np.asarray(shard.data)  # [BL, s_seg, U] f16 (blocks until ready)
    full[c * BL : (c + 1) * BL, s0 : s0 + s_seg] = arr  # f16 -> f32 cast


def run(x_codes, h0, timespans, weights, s_total=S, trace=False):
    import jax

    s_seg = min(SEG, s_total)
    assert s_total % s_seg == 0 and s_seg % CHUNK == 0
    nseg = s_total // s_seg
    rt = _get_rt(s_seg)
    blob_dev = _weights_dev(weights, rt["sh"])

    xf = np.asarray(x_codes, np.float32)[:, :s_total]
    xmax = float(max(xf.max(), -float(xf.min()), 1e-30))
    w0a_dev = _w0a_dev(weights, xmax, rt["sh"])
    sc = np.float32(127.0 / xmax)
    nts = -np.asarray(timespans, np.float32)[:, :s_total]
    h0 = np.asarray(h0, np.float32)
    h0T_g = np.ascontiguousarray(
        h0.reshape(NCORES, BL, U).transpose(0, 2, 1).reshape(NCORES * U, BL)
    )
    h_cur = jax.device_put(h0T_g, rt["sh"])

    by_name_static = {"blob": blob_dev, "w0a": w0a_dev}
    full = np.empty((B, s_total, U), np.float32)
    futs = []
    with ThreadPoolExecutor(max_workers=16) as ex:
        for si in range(nseg):
            s0 = si * s_seg
            xg = _prep_x_seg(xf, sc, s0, s_seg)
            ng = np.ascontiguousarray(nts[:, s0 : s0 + s_seg])
            x_dev = jax.device_put(xg, rt["sh"])
            n_dev = jax.device_put(ng, rt["sh"])
            by_name = {
                **by_name_static,
                "xT": x_dev, "nts": n_dev, "h0T": h_cur,
                "out": rt["ph_out"], "hTout": rt["ph_hT"],
            }
            args = [by_name[n] for n in rt["in_names"] + rt["out_names"]]
            res = rt["jitted"](*args)
            outs = dict(zip(rt["out_names"], res))
            h_cur = outs["hTout"]
            for shard in outs["out"].addressable_shards:
                futs.append(ex.submit(_fetch_shard, full, s0, s_seg, shard))
        for f in futs:
            f.result()
    return full, _Res()


def kernel(x_codes, h0, timespans, W0, b0, W1, b1, W2, b2, Wa, ba, Wb, bb):
    weights = _prep_weights(W0, b0, W1, b1, W2, b2, Wa, ba, Wb, bb)
    full, _ = run(
        np.asarray(x_codes, np.float32),
        np.asarray(h0, np.float32),
        np.asarray(timespans, np.float32),
        weights,
        S,
    )
    return full.astype(np.float32)
